# revision 1
# baseline (speedup 1.0000x reference)
"""Dilated (dil=2) 7x7 window self-attention, 4 heads x 32 dim, on 8 trn2 cores.

Strategy: spatial sharding over image rows (12 rows/core, 6-row halo).
Inside each core, the dilation-2 window decomposes the image into 4
cosets (row/col parity); within a coset the attention is a dense 7x7
window on a 48x48 grid.  All tensors are kept channel-major [128, pix];
logits are computed transposed [nk, nq] per (batch, coset) block so both
attention einsums are matmuls without any transposes:

  K^T Q  : 16-tile-packed 32x32 matmuls (per-head, reduction over d=32)
  softmax: unnormalized exp (no max-subtraction; logits are tiny) with
           the mask bias (-60 per masked key pixel) folded into the ACT
           exp bias; out-of-window pairs zeroed by one elementwise mul
           with a precomputed 0/1 window tensor; the softmax denominator
           comes from an extra ones-weight matmul pass and is divided
           out after attn@V.
  attn@V : col-tiled (4 heads) matmuls, reduction over nk chunks of 96,
           V produced directly in transposed [pix, ch] form by swapping
           the matmul operands of the V projection.
"""

import numpy as np

HEADS, D, WIN, DIL = 4, 32, 7, 2
B, C, H, W = 2, 128, 96, 96
CORES, RPC = 8, 12
CR, KR, W2 = 6, 12, 48            # coset query rows / key rows (halo) / cols
NQ, NK = CR * W2, KR * W2         # 288, 576
NBLK = B * 4                      # (batch, coset) blocks per core
SCALE = float(1.0 / np.sqrt(D))
MBIAS = -60.0

_prog = None


def _band32(c):
    """query-row band of 32-pixel key subchunk c (inclusive lo, hi)."""
    r_lo, r_hi = (32 * c) // W2, (32 * c + 31) // W2
    lo = max(0, r_lo - 6)
    hi = min(CR - 1, r_hi)
    return lo, hi


def _band(g):
    """query-row band of key-row pair {2g, 2g+1}: inclusive (lo, hi)."""
    rows = [i for i in range(CR)
            if (i <= 2 * g <= i + 6) or (i <= 2 * g + 1 <= i + 6)]
    return rows[0], rows[-1]


def _win_mask():
    """[NK, NQ] 0/1 in-window mask for one (batch, coset) block."""
    rr = np.arange(KR)[:, None, None, None]
    cc = np.arange(W2)[None, :, None, None]
    ii = np.arange(CR)[None, None, :, None]
    jj = np.arange(W2)[None, None, None, :]
    win = ((rr - ii >= 0) & (rr - ii <= 6) & (np.abs(cc - jj) <= 3))
    return win.reshape(NK, NQ).astype(np.float32)


def _build_program():
    import concourse.bass as bass
    import concourse.tile as tile
    from concourse import mybir

    nc = bass.Bass("TRN2", target_bir_lowering=False, debug=False,
                   num_devices=CORES)
    f32 = mybir.dt.float32
    mdt = mybir.dt.float32
    xc = nc.dram_tensor("xc", [128, NBLK * NK], f32, kind="ExternalInput").ap()
    mb_i = nc.dram_tensor("mb", [128, NBLK * 6], mybir.dt.int32,
                          kind="ExternalInput").ap()
    winm = nc.dram_tensor("winm", [128, 4 * 6 * NQ], f32,
                          kind="ExternalInput").ap()
    wq = nc.dram_tensor("wq", [128, 128], f32, kind="ExternalInput").ap()
    wk = nc.dram_tensor("wk", [128, 128], f32, kind="ExternalInput").ap()
    wv = nc.dram_tensor("wv", [128, 128], f32, kind="ExternalInput").ap()
    wp = nc.dram_tensor("wp", [128, 128], f32, kind="ExternalInput").ap()
    out = nc.dram_tensor("out", [128, NBLK * NQ], f32,
                         kind="ExternalOutput").ap()

    with tile.TileContext(nc) as tc:
        with tc.tile_pool(name="cst", bufs=1) as cst, \
             tc.tile_pool(name="big", bufs=1) as big, \
             tc.tile_pool(name="qk", bufs=1) as qkp, \
             tc.tile_pool(name="vt", bufs=2) as vtp, \
             tc.tile_pool(name="att", bufs=2) as attp, \
             tc.tile_pool(name="oev", bufs=3) as oev, \
             tc.tile_pool(name="psL", bufs=1, space="PSUM") as psL, \
             tc.tile_pool(name="psO", bufs=1, space="PSUM") as psO, \
             tc.tile_pool(name="psP", bufs=2, space="PSUM") as psP:

            w_q = cst.tile([128, 128], mdt)
            nc.gpsimd.dma_start(out=w_q[:], in_=wq[:])
            w_k = cst.tile([128, 128], mdt)
            nc.gpsimd.dma_start(out=w_k[:], in_=wk[:])
            w_v = cst.tile([128, 128], mdt)
            nc.gpsimd.dma_start(out=w_v[:], in_=wv[:])
            w_p = cst.tile([128, 128], mdt)
            nc.gpsimd.dma_start(out=w_p[:], in_=wp[:])

            X = big.tile([128, NBLK * NK], mdt)
            nc.gpsimd.dma_start(out=X[:], in_=xc[:])
            WM = big.tile([128, 4 * 6 * NQ], f32)   # win mask, coset-major
            nc.gpsimd.dma_start(out=WM[:], in_=winm[:])

            mbi = cst.tile([128, NBLK * 6], f32)
            mbraw = cst.tile([128, NBLK * 6], mybir.dt.int32)
            nc.gpsimd.dma_start(out=mbraw[:], in_=mb_i[:])
            nc.vector.tensor_copy(mbi[:], mbraw[:])        # int -> float
            mbias = cst.tile([128, NBLK * 6], f32)
            nc.vector.tensor_scalar(
                out=mbias[:], in0=mbi[:], scalar1=-MBIAS, scalar2=MBIAS,
                op0=mybir.AluOpType.mult, op1=mybir.AluOpType.add,
            )  # m*60 - 60 -> 0 (keep) / -60 (masked)

            pL0 = psL.tile([128, 2048], f32, tag="psL")
            nc.vector.memset(pL0[:], 0.0)

            ones_f = cst.tile([128, 32], f32)
            nc.vector.memset(ones_f[:], 1.0)
            ones = cst.tile([128, 32], mdt)
            nc.vector.tensor_copy(ones[:], ones_f[:])

            # Q and K channel-major projections for all blocks.
            Q = qkp.tile([128, NBLK * NQ], mdt)
            K = qkp.tile([128, NBLK * NK], mdt)
            for blk in range(NBLK):
                pq = psP.tile([128, 512], f32, tag="psP")
                nc.tensor.matmul(out=pq[:, :NQ], lhsT=w_q[:],
                                 rhs=X[:, blk * NK + 144: blk * NK + 144 + NQ],
                                 start=True, stop=True)
                if blk % 2:
                    nc.scalar.copy(out=Q[:, blk * NQ:(blk + 1) * NQ], in_=pq[:, :NQ])
                else:
                    nc.vector.tensor_copy(Q[:, blk * NQ:(blk + 1) * NQ], pq[:, :NQ])
                for half in range(2):
                    pk = psP.tile([128, 512], f32, tag="psP")
                    sl = slice(blk * NK + half * NQ, blk * NK + (half + 1) * NQ)
                    nc.tensor.matmul(out=pk[:, :NQ], lhsT=w_k[:], rhs=X[:, sl],
                                     start=True, stop=True)
                    if half:
                        nc.scalar.copy(out=K[:, sl], in_=pk[:, :NQ])
                    else:
                        nc.vector.tensor_copy(K[:, sl], pk[:, :NQ])

            for blk in range(NBLK):
                cs = blk % 4
                # --- V^T production: 6 chunks of 96 pixels ---
                VT = vtp.tile([128, 6 * 128], mdt, tag="vt")
                for pair in range(3):       # two 96-chunks per psum bank
                    pv = psP.tile([128, 512], f32, tag="psP")
                    for k2 in range(2):
                        g = pair * 2 + k2
                        nc.tensor.matmul(
                            out=pv[:96, k2 * 128:(k2 + 1) * 128],
                            lhsT=X[:, blk * NK + 96 * g:
                                   blk * NK + 96 * (g + 1)],
                            rhs=w_v[:], start=True, stop=True)
                    if pair % 2:
                        nc.scalar.copy(out=VT[:96, pair * 256:(pair + 1) * 256],
                                       in_=pv[:96, :256])
                    else:
                        nc.vector.tensor_copy(VT[:96, pair * 256:(pair + 1) * 256],
                                              pv[:96, :256])

                # --- phase 1 + exp + window mask ---
                attnT = attp.tile([128, 4 * 6 * NQ], mdt, tag="att")
                for g in range(6):
                    lo, hi = _band(g)
                    nlo, nn = lo * W2, (hi - lo + 1) * W2
                    pL = psL.tile([128, 2048], f32, tag="psL")
                    for k3 in range(3):
                        c32 = 3 * g + k3
                        lo3, hi3 = _band32(c32)
                        n3, nn3 = lo3 * W2, (hi3 - lo3 + 1) * W2
                        for h in range(4):
                            nc.tensor.matmul(
                                out=pL[32 * k3:32 * k3 + 32,
                                       512 * h + n3: 512 * h + n3 + nn3],
                                lhsT=K[32 * h:32 * h + 32,
                                       blk * NK + 32 * c32:
                                       blk * NK + 32 * c32 + 32].bitcast(f32),
                                rhs=Q[32 * h:32 * h + 32,
                                      blk * NQ + n3:
                                      blk * NQ + n3 + nn3].bitcast(f32),
                                start=True, stop=True,
                                tile_position=(32 * h, 32 * k3),
                            )
                    # exp over 4 heads at once: AP [96, (4 banks, nn)]
                    src = pL[:96].rearrange("p (h n) -> p h n", h=4)[:, :, nlo:nlo + nn]
                    dst = attnT[:96].rearrange("p (h g n) -> p h g n", h=4, g=6)[:, :, g, nlo:nlo + nn]
                    nc.scalar.activation(
                        out=dst, in_=src,
                        func=mybir.ActivationFunctionType.Exp,
                        bias=mbias[0:96, blk * 6 + g: blk * 6 + g + 1],
                        scale=SCALE,
                    )
                    # zero out-of-window pairs (win==0) and garbage rows
                    wsrc = WM[0:96, cs * 6 * NQ + g * NQ + nlo:
                              cs * 6 * NQ + g * NQ + nlo + nn]
                    for h in range(4):
                        dsth = attnT[0:96, (h * 6 + g) * NQ + nlo:
                                     (h * 6 + g) * NQ + nlo + nn]
                        eng = nc.vector if h % 2 else nc.gpsimd
                        eng.tensor_mul(out=dsth, in0=dsth, in1=wsrc)

                # --- phase 2 (attn @ V^T) + rowsum, col-tiled by head ---
                pO = psO.tile([128, 512], f32, tag="psO")
                pS = psO.tile([128, 512], f32, tag="psS")
                for g in range(6):
                    lo, hi = _band(g)
                    nlo, nn = lo * W2, (hi - lo + 1) * W2
                    for h in range(4):
                        rhs = attnT[0:96, (h * 6 + g) * NQ + nlo:
                                    (h * 6 + g) * NQ + nlo + nn].bitcast(f32)
                        nc.tensor.matmul(
                            out=pO[32 * h:32 * h + 32, nlo:nlo + nn],
                            lhsT=VT[0:96, g * 128 + 32 * h:
                                    g * 128 + 32 * h + 32].bitcast(f32),
                            rhs=rhs, start=(g == 0), stop=(g == 5),
                            tile_position=(0, 32 * h),
                        )
                        nc.tensor.matmul(
                            out=pS[32 * h:32 * h + 32, nlo:nlo + nn],
                            lhsT=ones[0:96, :].bitcast(f32),
                            rhs=rhs, start=(g == 0), stop=(g == 5),
                            tile_position=(0, 32 * h),
                        )
                rcp = oev.tile([128, NQ], f32, tag="rcp")
                nc.vector.reciprocal(out=rcp[:], in_=pS[:, :NQ])
                onrm = oev.tile([128, NQ], mdt, tag="onrm")
                nc.vector.tensor_mul(out=onrm[:], in0=pO[:, :NQ], in1=rcp[:])

                # --- final projection ---
                pF = psP.tile([128, 512], f32, tag="psP")
                nc.tensor.matmul(out=pF[:, :NQ], lhsT=w_p[:], rhs=onrm[:],
                                 start=True, stop=True)
                osb = oev.tile([128, NQ], f32, tag="osb")
                nc.scalar.copy(out=osb[:], in_=pF[:, :NQ])
                nc.gpsimd.dma_start(out=out[:, blk * NQ:(blk + 1) * NQ],
                                    in_=osb[:])

    _split_multi_waits(nc)
    return nc


def _split_multi_waits(nc):
    """This walrus build rejects >1 sem wait per instruction: move extra
    waits onto dedicated single-wait NoOps inserted just before."""
    import copy
    from concourse import mybir

    tmpl = nc.sync.nop(nofuse=True, hint="wsplit_template").ins
    bb0 = nc.cur_bb.bb
    bb0.instructions = [i for i in bb0.instructions if i.name != tmpl.name]
    tmpl = copy.deepcopy(tmpl)

    ctr = 0
    for f in nc.m.functions:
        for bb in f.blocks:
            insts = list(bb.instructions)
            new, changed = [], False
            for inst in insts:
                si = getattr(inst, "sync_info", None)
                waits = list(si.on_wait) if si is not None and si.on_wait else []
                if len(waits) > 1:
                    for w in waits[:-1]:
                        ctr += 1
                        nop = copy.deepcopy(tmpl)
                        nop.name = f"I-wsplit{ctr}"
                        nop.engine = inst.engine
                        nop.sync_info = mybir.SyncInfo(on_wait=[w], on_update=[])
                        new.append(nop)
                    si.on_wait = [waits[-1]]
                    changed = True
                new.append(inst)
            if changed:
                bb.instructions = new


def _host_prep(x, m):
    xs, ms = [], []
    for k in range(CORES):
        r0 = 12 * k - 6
        xpad = np.zeros((B, C, 24, W), np.float32)
        mpad = np.zeros((B, 1, 24, W), np.int32)
        lo, hi = max(0, r0), min(H, r0 + 24)
        xpad[:, :, lo - r0:hi - r0] = x[:, :, lo:hi]
        mpad[:, :, lo - r0:hi - r0] = m[:, :, lo:hi]
        xcs = xpad.reshape(B, C, KR, 2, W2, 2).transpose(1, 0, 3, 5, 2, 4)
        xcs = np.ascontiguousarray(xcs.reshape(C, NBLK * NK))
        mc = mpad.reshape(B, 1, KR, 2, W2, 2).transpose(1, 0, 3, 5, 2, 4)
        mc = mc.reshape(B, 4, NK)
        mb = np.ones((128, NBLK * 6), np.int32)
        for b in range(B):
            for cspar in range(4):
                for g in range(6):
                    mb[:96, (b * 4 + cspar) * 6 + g] = \
                        mc[b, cspar, 96 * g:96 * (g + 1)]
        xs.append(xcs)
        ms.append(np.ascontiguousarray(mb))
    return xs, ms


def _host_win():
    """[128, 4*6*NQ]: win mask in attnT layout (heads=4 share; here the
    '4' axis is heads, identical; partitions 96-127 zero)."""
    win = _win_mask()                        # [NK, NQ]
    wm = np.zeros((128, 4, 6, NQ), np.float32)
    for g in range(6):
        wm[:96, :, g, :] = win[96 * g:96 * (g + 1), None, :]
    return np.ascontiguousarray(wm.reshape(128, 4 * 6 * NQ))


def kernel(x, m, Wq, Wk, Wv, Wp):
    global _prog
    from concourse.bass_utils import run_bass_kernel_spmd

    x = np.asarray(x, dtype=np.float32)
    m = np.asarray(m, dtype=np.int32)
    if _prog is None:
        _prog = _build_program()
    nc = _prog

    xs, ms = _host_prep(x, m)
    wmask = _host_win()
    base = {
        "winm": wmask,
        "wq": np.ascontiguousarray(np.asarray(Wq, np.float32).T),
        "wk": np.ascontiguousarray(np.asarray(Wk, np.float32).T),
        "wv": np.ascontiguousarray(np.asarray(Wv, np.float32).T),
        "wp": np.ascontiguousarray(np.asarray(Wp, np.float32).T),
    }
    in_maps = [{**base, "xc": xs[k], "mb": ms[k]} for k in range(CORES)]
    res = run_bass_kernel_spmd(nc, in_maps, list(range(CORES)))

    full = np.zeros((B, C, H, W), np.float32)
    for k in range(CORES):
        oc = res.results[k]["out"].reshape(C, B, 2, 2, CR, W2)
        o = oc.transpose(1, 0, 4, 2, 5, 3).reshape(B, C, 12, 96)
        full[:, :, 12 * k:12 * k + 12, :] = o
    return full



# revision 25
# speedup vs baseline: 2.7380x; 2.7380x over previous
"""Dilated (dil=2) 7x7 window self-attention, 4 heads x 32 dim, on 8 trn2 cores.

v2: spatial sharding over image rows (12 rows/core, 6-row halo), 4 cosets
(row/col parity) x 2 batches = 8 independent blocks per core.  Within a
block the coset grid is 6 query rows x 48 cols (NQ=288) attending over
12 key rows x 48 cols (NK=576) with a dense 7x7 window (|dr|,|dc| <= 3
in coset space; local key row kr attends query rows qr in [kr-6, kr]).

All matmuls bf16 (tolerance 2e-2 gives plenty of slack):
  - keys split into 6 column-chunks of 8 cols (96 keys = 12r x 8c each);
    queries touched by chunk j = 6 rows x 14 cols (global cols 8j-3..
    8j+10, clipped) -> logits unit [96 keys, 6x14=84] per (chunk, head).
  - phase 1: one matmul per (chunk, head): lhsT = K chunk [32, 96],
    rhs = Q window [32, 6, w] -> psum unit; 4 heads packed via
    tile_position rows.  24 units = 4 psum banks (6 units x 84 per bank).
  - exp: one ACT instruction per 2-bank half (12 units), no bias, no max
    subtraction (logits are tiny); writes bf16 attnT.
  - window mask: one bf16 multiply per half with a precomputed 0/1 mask
    (same for every unit).
  - key masking: V is projected from host-premultiplied x*m, so masked
    and padding keys contribute 0 to the numerator; the denominator is a
    matmul with lhsT = per-(block,chunk) key validity (eps for invalid)
    replicated x32, so invalid keys contribute ~eps.
  - phase 2: per chunk, 4 pO + 4 pS matmuls (col-tiled by head),
    accumulated across chunks into overlapping [32h, 6, w] psum windows.
  - normalize: reciprocal_approx_fast(pS) * pO -> bf16, then the 1x1
    output projection and a psum->sbuf fp32 copy + DMA out.

Blocks are software-pipelined: projections of block b+1 are emitted
between phase 1 and phase 2 of block b so the PE never waits on the
ACT/DVE exp/mask chain.
"""

import numpy as np

HEADS, D, WIN, DIL = 4, 32, 7, 2
B, C, H, W = 2, 128, 96, 96
CORES = 8
CR, KR, W2 = 6, 12, 48            # coset query rows / key rows (halo) / cols
NQ, NK = CR * W2, KR * W2         # 288, 576
NBLK = B * 4                      # (batch, coset) blocks per core
NCH = 6                           # key column chunks of 8
SCALE = float(1.0 / np.sqrt(D))
EPS = 1e-5                        # denominator weight for invalid keys
_PIPE = True                      # software-pipeline blocks
_NRUN = NBLK                      # blocks to emit in no-pipe debug mode
_STAGES = 5                       # no-pipe debug: how many stages to emit

_prog = None


def _chunk_geo(j):
    """(gl0, w, l0): global q-col start, width, offset in 14-col frame."""
    gl0 = max(0, 8 * j - 3)
    gl1 = min(W2 - 1, 8 * j + 10)
    return gl0, gl1 - gl0 + 1, gl0 - (8 * j - 3)


def _unit_off(j, h):
    """attnT / psum offsets of unit (chunk j, head h).  Bank h holds head
    h's six 84-wide units — concurrent head-tiles must drain to DISTINCT
    psum banks (same-bank same-partition concurrent drains fault the HW)."""
    att = h * 504 + j * 84
    pl = h * 512 + j * 84
    return att, pl


def _build_program():
    import concourse.bass as bass
    import concourse.tile as tile
    from concourse import mybir

    nc = bass.Bass("TRN2", target_bir_lowering=False, debug=False,
                   num_devices=CORES)
    f32 = mybir.dt.float32
    bf16 = mybir.dt.bfloat16

    xq_d = nc.dram_tensor("xq", [128, NBLK * NQ], bf16, kind="ExternalInput").ap()
    xk_d = nc.dram_tensor("xk", [128, NBLK * NK], bf16, kind="ExternalInput").ap()
    xm_d = nc.dram_tensor("xm", [128, NBLK * NK], bf16, kind="ExternalInput").ap()
    wm_d = nc.dram_tensor("wm", [128, 1008], bf16, kind="ExternalInput").ap()
    mk_d = nc.dram_tensor("mk", [128, NBLK * NCH * 32], bf16,
                          kind="ExternalInput").ap()
    wq_d = nc.dram_tensor("wq", [128, 128], bf16, kind="ExternalInput").ap()
    wk_d = nc.dram_tensor("wk", [128, 128], bf16, kind="ExternalInput").ap()
    wv_d = nc.dram_tensor("wv", [128, 128], bf16, kind="ExternalInput").ap()
    wp_d = nc.dram_tensor("wp", [128, 128], bf16, kind="ExternalInput").ap()
    out_d = nc.dram_tensor("out", [128, NBLK * NQ], f32,
                           kind="ExternalOutput").ap()

    with tile.TileContext(nc) as tc:
        with tc.tile_pool(name="cst", bufs=1) as cst, \
             tc.tile_pool(name="qk", bufs=2) as qkp, \
             tc.tile_pool(name="vt", bufs=3) as vtp, \
             tc.tile_pool(name="att", bufs=2) as attp, \
             tc.tile_pool(name="nrm", bufs=2) as nrm, \
             tc.tile_pool(name="psL", bufs=1, space="PSUM") as psL, \
             tc.tile_pool(name="psO", bufs=1, space="PSUM") as psO, \
             tc.tile_pool(name="psP", bufs=2, space="PSUM") as psP:

            w_q = cst.tile([128, 128], bf16)
            nc.gpsimd.dma_start(out=w_q[:], in_=wq_d[:])
            w_k = cst.tile([128, 128], bf16)
            nc.gpsimd.dma_start(out=w_k[:], in_=wk_d[:])
            w_v = cst.tile([128, 128], bf16)
            nc.gpsimd.dma_start(out=w_v[:], in_=wv_d[:])
            w_p = cst.tile([128, 128], bf16)
            nc.gpsimd.dma_start(out=w_p[:], in_=wp_d[:])
            WMt = cst.tile([128, 1008], bf16)
            nc.gpsimd.dma_start(out=WMt[:], in_=wm_d[:])
            MKt = cst.tile([128, NBLK * NCH * 32], bf16)
            nc.gpsimd.dma_start(out=MKt[:], in_=mk_d[:])

            Xq = cst.tile([128, NBLK * NQ], bf16)
            Xk = cst.tile([128, NBLK * NK], bf16)
            Xm = cst.tile([128, NBLK * NK], bf16)
            for q in range(4):
                sl = slice(q * NBLK * NK // 4, (q + 1) * NBLK * NK // 4)
                nc.gpsimd.dma_start(out=Xk[:, sl], in_=xk_d[:, sl])
                nc.gpsimd.dma_start(out=Xm[:, sl], in_=xm_d[:, sl])
                s2 = slice(q * NBLK * NQ // 4, (q + 1) * NBLK * NQ // 4)
                nc.gpsimd.dma_start(out=Xq[:, s2], in_=xq_d[:, s2])

            # zero the psL ring slot once so exp of never-written lanes
            # stays bounded
            plz0 = psL.tile([128, 2048], f32, tag="pl")
            nc.vector.memset(plz0[:], 0.0)

            # per-block state carried between pipeline stages
            st = [dict() for _ in range(NBLK)]

            def proj(b):
                s = st[b]
                xkb = Xk[:, b * NK:(b + 1) * NK]

                Qb = qkp.tile([128, NQ], bf16, tag="q", name=f"Qb{b}")
                Kb = qkp.tile([128, NK], bf16, tag="k", name=f"Kb{b}")
                VTb = vtp.tile([128, NCH * 128], bf16, tag="vt", name=f"VTb{b}")

                pq = psP.tile([128, 512], f32, tag="pp", name=f"pq{b}")
                nc.tensor.matmul(out=pq[:, :NQ], lhsT=w_q[:],
                                 rhs=Xq[:, b * NQ:(b + 1) * NQ],
                                 start=True, stop=True)
                nc.scalar.copy(out=Qb[:], in_=pq[:, :NQ])

                for half in range(2):
                    pk = psP.tile([128, 512], f32, tag="pp", name=f"pk{b}_{half}")
                    nc.tensor.matmul(out=pk[:, :NQ], lhsT=w_k[:],
                                     rhs=xkb[:, half * NQ:(half + 1) * NQ],
                                     start=True, stop=True)
                    if half:
                        nc.scalar.copy(out=Kb[:, NQ:], in_=pk[:, :NQ])
                    else:
                        nc.vector.tensor_copy(Kb[:, :NQ], pk[:, :NQ])

                pv1 = psP.tile([128, 512], f32, tag="pp", name=f"pv1{b}")
                for j in range(4):
                    nc.tensor.matmul(out=pv1[0:96, j * 128:(j + 1) * 128],
                                     lhsT=Xm[:, (b * NCH + j) * 96:
                                             (b * NCH + j + 1) * 96],
                                     rhs=w_v[:], start=True, stop=True)
                pv2 = psP.tile([128, 512], f32, tag="pp", name=f"pv2{b}")
                for j in range(4, 6):
                    nc.tensor.matmul(out=pv2[0:96, (j - 4) * 128:(j - 3) * 128],
                                     lhsT=Xm[:, (b * NCH + j) * 96:
                                             (b * NCH + j + 1) * 96],
                                     rhs=w_v[:], start=True, stop=True)
                nc.vector.tensor_copy(VTb[0:96, :512], pv1[0:96, :])
                nc.vector.tensor_copy(VTb[0:96, 512:768], pv2[0:96, :256])
                s["Q"], s["K"], s["VT"] = Qb, Kb, VTb

            def ph1(b):
                s = st[b]
                pl = psL.tile([128, 2048], mybir.dt.float32, tag="pl",
                              name=f"pl{b}")
                s["pl"] = pl
                for j in range(NCH):
                    gl0, w, l0 = _chunk_geo(j)
                    for h in range(4):
                        _, ploff = _unit_off(j, h)
                        dst = pl[0:96, ploff:ploff + 84] \
                            .rearrange("p (r c) -> p r c", c=14)[:, :, l0:l0 + w]
                        lhsT = s["K"][32 * h:32 * h + 32,
                                      j * 96:(j + 1) * 96]
                        rhs = s["Q"][32 * h:32 * h + 32, :] \
                            .rearrange("p (r c) -> p r c", c=W2)[:, :, gl0:gl0 + w]
                        nc.tensor.matmul(out=dst, lhsT=lhsT, rhs=rhs,
                                         start=True, stop=True,
                                         tile_position=(32 * h, 0))

            def expmask(b):
                s = st[b]
                att = attp.tile([128, 2016], mybir.dt.bfloat16,
                                tag="att", name=f"att{b}")
                s["att"] = att
                src = s["pl"][0:96, :].rearrange("p (k x) -> p k x",
                                                 k=4)[:, :, 0:504]
                dst = att[0:96, :].rearrange("p (k x) -> p k x", k=4)
                nc.scalar.activation(out=dst, in_=src,
                                     func=mybir.ActivationFunctionType.Exp,
                                     scale=SCALE)
                if _STAGES >= 2:
                    for half in range(2):
                        sl = slice(half * 1008, (half + 1) * 1008)
                        nc.vector.tensor_mul(out=att[0:96, sl],
                                             in0=att[0:96, sl],
                                             in1=WMt[0:96, 0:1008])

            def ph2(b, jlist):
                s = st[b]
                if "pO" not in s:
                    s["pO"] = psO.tile([128, 512], mybir.dt.float32, tag="po",
                                       name=f"pO{b}")
                    s["pS"] = psO.tile([128, 512], mybir.dt.float32, tag="ps",
                                       name=f"pS{b}")
                pO, pS = s["pO"], s["pS"]
                for j in jlist:
                    gl0, w, l0 = _chunk_geo(j)
                    for h in range(4):
                        attoff, _ = _unit_off(j, h)
                        rhs = s["att"][0:96, attoff:attoff + 84] \
                            .rearrange("p (r c) -> p r c", c=14)[:, :, l0:l0 + w]
                        dstO = pO[32 * h:32 * h + 32, :NQ] \
                            .rearrange("p (r c) -> p r c", c=W2)[:, :, gl0:gl0 + w]
                        nc.tensor.matmul(
                            out=dstO, lhsT=s["VT"][0:96, j * 128 + 32 * h:
                                                   j * 128 + 32 * h + 32],
                            rhs=rhs, start=(j == 0), stop=(j == 5),
                            tile_position=(0, 32 * h))
                        dstS = pS[32 * h:32 * h + 32, :NQ] \
                            .rearrange("p (r c) -> p r c", c=W2)[:, :, gl0:gl0 + w]
                        nc.tensor.matmul(
                            out=dstS,
                            lhsT=MKt[0:96, (b * NCH + j) * 32:
                                     (b * NCH + j) * 32 + 32],
                            rhs=rhs, start=(j == 0), stop=(j == 5),
                            tile_position=(0, 32 * h))

            def norm(b):
                s = st[b]
                rcp = nrm.tile([128, NQ], mybir.dt.float32, tag="rcp",
                               name=f"rcp{b}")
                nc.vector.reciprocal(out=rcp[:], in_=s["pS"][:, :NQ])
                onrm = nrm.tile([128, NQ], mybir.dt.bfloat16, tag="on",
                                name=f"on{b}")
                nc.vector.tensor_mul(out=onrm[:], in0=s["pO"][:, :NQ], in1=rcp[:])
                s["on"] = onrm

            def final(b):
                s = st[b]
                pf = psP.tile([128, 512], mybir.dt.float32, tag="pp",
                              name=f"pf{b}")
                nc.tensor.matmul(out=pf[:, :NQ], lhsT=w_p[:], rhs=s["on"][:],
                                 start=True, stop=True)
                osb = nrm.tile([128, NQ], mybir.dt.float32, tag="osb",
                               name=f"osb{b}")
                nc.scalar.copy(out=osb[:], in_=pf[:, :NQ])
                nc.gpsimd.dma_start(out=out_d[:, b * NQ:(b + 1) * NQ],
                                    in_=osb[:])
                st[b] = {}

            if not _PIPE:
                for b in range(_NRUN):
                    proj(b)
                    if _STAGES >= 1.3:
                        ph1(b)
                    if _STAGES >= 1.6:
                        expmask(b)
                    if _STAGES >= 3:
                        ph2(b, [0, 1, 2])
                        ph2(b, [3, 4, 5])
                    if _STAGES >= 4:
                        norm(b)
                    if _STAGES >= 5:
                        final(b)
            else:
                # 4-stage software pipeline per iteration `it`:
                #   proj(it+1) | ph1/exp/mask(it) | ph2/norm(it-1) | final(it-2)
                # so the PE never waits on the ACT exp / DVE mask chain.
                proj(0)
                for it in range(NBLK + 2):
                    if it + 1 < NBLK:
                        proj(it + 1)
                    if it < NBLK:
                        ph1(it)
                        expmask(it)
                    if 0 <= it - 1 < NBLK:
                        ph2(it - 1, [0, 1, 2])
                        ph2(it - 1, [3, 4, 5])
                        norm(it - 1)
                    if 0 <= it - 2 < NBLK:
                        final(it - 2)

    _split_multi_waits(nc)
    return nc


def _split_multi_waits(nc):
    """This walrus build rejects >1 sem wait per instruction: move extra
    waits onto dedicated single-wait NoOps inserted just before."""
    import copy
    from concourse import mybir

    tmpl = nc.sync.nop(nofuse=True, hint="wsplit_template").ins
    bb0 = nc.cur_bb.bb
    bb0.instructions = [i for i in bb0.instructions if i.name != tmpl.name]
    tmpl = copy.deepcopy(tmpl)

    ctr = 0
    for f in nc.m.functions:
        for bb in f.blocks:
            insts = list(bb.instructions)
            new, changed = [], False
            for inst in insts:
                si = getattr(inst, "sync_info", None)
                waits = list(si.on_wait) if si is not None and si.on_wait else []
                if len(waits) > 1:
                    for w in waits[:-1]:
                        ctr += 1
                        nop = copy.deepcopy(tmpl)
                        nop.name = f"I-wsplit{ctr}"
                        nop.engine = inst.engine
                        nop.sync_info = mybir.SyncInfo(on_wait=[w], on_update=[])
                        new.append(nop)
                    si.on_wait = [waits[-1]]
                    changed = True
                new.append(inst)
            if changed:
                bb.instructions = new


def _host_prep(x, m):
    """Per-core inputs: xq [128, NBLK*NQ] row-major center rows; xk/xm
    [128, NBLK*NK] chunk-major (key p = (j, kr, kc')); mk [128, NBLK*6*32]."""
    import ml_dtypes
    bf = ml_dtypes.bfloat16
    # chunk-major permutation of a 576-key block
    perm = np.array([kr * W2 + 8 * j + kc
                     for j in range(NCH) for kr in range(KR)
                     for kc in range(8)], np.int64)
    xqs, xks, xms, mks = [], [], [], []
    mf = (m > 0).astype(np.float32)
    for k in range(CORES):
        r0 = 12 * k - 6
        xpad = np.zeros((B, C, 24, W), np.float32)
        mpad = np.zeros((B, 1, 24, W), np.float32)
        lo, hi = max(0, r0), min(H, r0 + 24)
        xpad[:, :, lo - r0:hi - r0] = x[:, :, lo:hi]
        mpad[:, :, lo - r0:hi - r0] = mf[:, :, lo:hi]
        xmp = xpad * mpad

        def coset(t, ch):
            v = t.reshape(B, ch, KR, 2, W2, 2).transpose(1, 0, 3, 5, 2, 4)
            return v.reshape(ch, NBLK, NK)

        xc = coset(xpad, C)                       # [C, NBLK, NK] row-major
        xq = xc[:, :, 144:144 + NQ].reshape(C, NBLK * NQ)
        xk = xc[:, :, perm].reshape(C, NBLK * NK)
        xm = coset(xmp, C)[:, :, perm].reshape(C, NBLK * NK)
        xqs.append(np.ascontiguousarray(xq).astype(bf))
        xks.append(np.ascontiguousarray(xk).astype(bf))
        xms.append(np.ascontiguousarray(xm).astype(bf))
        mc = coset(mpad, 1)[0][:, perm].reshape(NBLK, NCH, 96)
        mk = np.zeros((128, NBLK * NCH * 32), np.float32)
        vals = np.where(mc > 0, 1.0, EPS)         # [NBLK, NCH, 96]
        mk[0:96] = np.repeat(vals.reshape(NBLK * NCH, 96).T, 32, axis=1)
        mks.append(mk.astype(bf))
    return xqs, xks, xms, mks


def _host_wm():
    """[128, 1008] bf16: 0/1 window mask, unit layout [12 units][6 qr][14 lc],
    key partition p = kr*8 + kc'."""
    import ml_dtypes
    kr = np.arange(KR)[:, None, None, None]
    kc = np.arange(8)[None, :, None, None]
    qr = np.arange(CR)[None, None, :, None]
    lc = np.arange(14)[None, None, None, :]
    win = ((kr - qr >= 0) & (kr - qr <= 6) & (lc >= kc) & (lc <= kc + 6))
    unit = win.reshape(96, 84).astype(np.float32)
    wm = np.zeros((128, 1008), np.float32)
    wm[0:96] = np.tile(unit, (1, 12))
    return wm.astype(ml_dtypes.bfloat16)


def _make_in_maps(x, m, Wq, Wk, Wv, Wp):
    import ml_dtypes
    bf = ml_dtypes.bfloat16
    xqs, xks, xms, mks = _host_prep(np.asarray(x, np.float32),
                                    np.asarray(m, np.int32))
    base = {
        "wm": _host_wm(),
        "wq": np.ascontiguousarray(np.asarray(Wq, np.float32).T).astype(bf),
        "wk": np.ascontiguousarray(np.asarray(Wk, np.float32).T).astype(bf),
        "wv": np.ascontiguousarray(np.asarray(Wv, np.float32).T).astype(bf),
        "wp": np.ascontiguousarray(np.asarray(Wp, np.float32).T).astype(bf),
    }
    return [{**base, "xq": xqs[k], "xk": xks[k], "xm": xms[k], "mk": mks[k]}
            for k in range(CORES)]


def kernel(x, m, Wq, Wk, Wv, Wp):
    global _prog
    from concourse.bass_utils import run_bass_kernel_spmd

    if _prog is None:
        _prog = _build_program()

    in_maps = _make_in_maps(x, m, Wq, Wk, Wv, Wp)
    res = run_bass_kernel_spmd(_prog, in_maps, list(range(CORES)))

    full = np.zeros((B, C, H, W), np.float32)
    for k in range(CORES):
        oc = res.results[k]["out"].reshape(C, B, 2, 2, CR, W2)
        o = oc.transpose(1, 0, 4, 2, 5, 3).reshape(B, C, 12, 96)
        full[:, :, 12 * k:12 * k + 12, :] = o
    return full


# revision 31
# speedup vs baseline: 2.8091x; 1.0260x over previous
"""Dilated (dil=2) 7x7 window self-attention, 4 heads x 32 dim, on 8 trn2 cores.

v2: spatial sharding over image rows (12 rows/core, 6-row halo), 4 cosets
(row/col parity) x 2 batches = 8 independent blocks per core.  Within a
block the coset grid is 6 query rows x 48 cols (NQ=288) attending over
12 key rows x 48 cols (NK=576) with a dense 7x7 window (|dr|,|dc| <= 3
in coset space; local key row kr attends query rows qr in [kr-6, kr]).

All matmuls bf16 (tolerance 2e-2 gives plenty of slack):
  - keys split into 6 column-chunks of 8 cols (96 keys = 12r x 8c each);
    queries touched by chunk j = 6 rows x 14 cols (global cols 8j-3..
    8j+10, clipped) -> logits unit [96 keys, 6x14=84] per (chunk, head).
  - phase 1: one matmul per (chunk, head): lhsT = K chunk [32, 96],
    rhs = Q window [32, 6, w] -> psum unit; 4 heads packed via
    tile_position rows.  24 units = 4 psum banks (6 units x 84 per bank).
  - exp: one ACT instruction per 2-bank half (12 units), no bias, no max
    subtraction (logits are tiny); writes bf16 attnT.
  - window mask: one bf16 multiply per half with a precomputed 0/1 mask
    (same for every unit).
  - key masking: V is projected from host-premultiplied x*m, so masked
    and padding keys contribute 0 to the numerator; the denominator is a
    matmul with lhsT = per-(block,chunk) key validity (eps for invalid)
    replicated x32, so invalid keys contribute ~eps.
  - phase 2: per chunk, 4 pO + 4 pS matmuls (col-tiled by head),
    accumulated across chunks into overlapping [32h, 6, w] psum windows.
  - normalize: reciprocal_approx_fast(pS) * pO -> bf16, then the 1x1
    output projection and a psum->sbuf fp32 copy + DMA out.

Blocks are software-pipelined: projections of block b+1 are emitted
between phase 1 and phase 2 of block b so the PE never waits on the
ACT/DVE exp/mask chain.
"""

import numpy as np

HEADS, D, WIN, DIL = 4, 32, 7, 2
B, C, H, W = 2, 128, 96, 96
CORES = 8
CR, KR, W2 = 6, 12, 48            # coset query rows / key rows (halo) / cols
NQ, NK = CR * W2, KR * W2         # 288, 576
NBLK = B * 4                      # (batch, coset) blocks per core
NCH = 6                           # key column chunks of 8
SCALE = float(1.0 / np.sqrt(D))
EPS = 1e-5                        # denominator weight for invalid keys
_PIPE = True                      # software-pipeline blocks
_NRUN = NBLK                      # blocks to emit in no-pipe debug mode
_STAGES = 5                       # no-pipe debug: how many stages to emit

_prog = None


def _chunk_geo(j):
    """(gl0, w, l0): global q-col start, width, offset in 14-col frame."""
    gl0 = max(0, 8 * j - 3)
    gl1 = min(W2 - 1, 8 * j + 10)
    return gl0, gl1 - gl0 + 1, gl0 - (8 * j - 3)


def _unit_off(j, h):
    """attnT / psum offsets of unit (chunk j, head h).  Bank h holds head
    h's six 84-wide units — concurrent head-tiles must drain to DISTINCT
    psum banks (same-bank same-partition concurrent drains fault the HW)."""
    att = h * 504 + j * 84
    pl = h * 512 + j * 84
    return att, pl


def _build_program():
    import concourse.bass as bass
    import concourse.tile as tile
    from concourse import mybir

    nc = bass.Bass("TRN2", target_bir_lowering=False, debug=False,
                   num_devices=CORES)
    f32 = mybir.dt.float32
    bf16 = mybir.dt.bfloat16

    xq_d = nc.dram_tensor("xq", [128, NBLK * NQ], bf16, kind="ExternalInput").ap()
    xk_d = nc.dram_tensor("xk", [128, NBLK * NK], bf16, kind="ExternalInput").ap()
    xm_d = nc.dram_tensor("xm", [128, NBLK * NK], bf16, kind="ExternalInput").ap()
    wm_d = nc.dram_tensor("wm", [128, 1008], bf16, kind="ExternalInput").ap()
    mk_d = nc.dram_tensor("mk", [128, NBLK * NCH * 32], bf16,
                          kind="ExternalInput").ap()
    wq_d = nc.dram_tensor("wq", [128, 128], bf16, kind="ExternalInput").ap()
    wk_d = nc.dram_tensor("wk", [128, 128], bf16, kind="ExternalInput").ap()
    wv_d = nc.dram_tensor("wv", [128, 128], bf16, kind="ExternalInput").ap()
    wp_d = nc.dram_tensor("wp", [128, 128], bf16, kind="ExternalInput").ap()
    out_d = nc.dram_tensor("out", [128, NBLK * NQ], f32,
                           kind="ExternalOutput").ap()

    with tile.TileContext(nc) as tc:
        with tc.tile_pool(name="cst", bufs=1) as cst, \
             tc.tile_pool(name="qk", bufs=2) as qkp, \
             tc.tile_pool(name="vt", bufs=3) as vtp, \
             tc.tile_pool(name="att", bufs=2) as attp, \
             tc.tile_pool(name="nrm", bufs=2) as nrm, \
             tc.tile_pool(name="psL", bufs=1, space="PSUM") as psL, \
             tc.tile_pool(name="psO", bufs=1, space="PSUM") as psO, \
             tc.tile_pool(name="psP", bufs=2, space="PSUM") as psP:

            w_q = cst.tile([128, 128], bf16)
            nc.gpsimd.dma_start(out=w_q[:], in_=wq_d[:])
            w_k = cst.tile([128, 128], bf16)
            nc.gpsimd.dma_start(out=w_k[:], in_=wk_d[:])
            w_v = cst.tile([128, 128], bf16)
            nc.gpsimd.dma_start(out=w_v[:], in_=wv_d[:])
            w_p = cst.tile([128, 128], bf16)
            nc.gpsimd.dma_start(out=w_p[:], in_=wp_d[:])
            WMt = cst.tile([128, 1008], bf16)
            nc.gpsimd.dma_start(out=WMt[:], in_=wm_d[:])
            MKt = cst.tile([128, NBLK * NCH * 32], bf16)
            nc.gpsimd.dma_start(out=MKt[:], in_=mk_d[:])

            Xq = cst.tile([128, NBLK * NQ], bf16)
            Xk = cst.tile([128, NBLK * NK], bf16)
            Xm = cst.tile([128, NBLK * NK], bf16)
            for q in range(4):
                sl = slice(q * NBLK * NK // 4, (q + 1) * NBLK * NK // 4)
                nc.gpsimd.dma_start(out=Xk[:, sl], in_=xk_d[:, sl])
                nc.gpsimd.dma_start(out=Xm[:, sl], in_=xm_d[:, sl])
                s2 = slice(q * NBLK * NQ // 4, (q + 1) * NBLK * NQ // 4)
                nc.gpsimd.dma_start(out=Xq[:, s2], in_=xq_d[:, s2])

            # zero the psL ring slots once so exp of never-written lanes
            # stays bounded
            plz0 = psL.tile([128, 1024], f32, tag="plA")
            nc.vector.memset(plz0[:], 0.0)
            plz1 = psL.tile([128, 1024], f32, tag="plB")
            nc.vector.memset(plz1[:], 0.0)

            # per-block state carried between pipeline stages
            st = [dict() for _ in range(NBLK)]

            def proj(b):
                s = st[b]
                xkb = Xk[:, b * NK:(b + 1) * NK]

                Qb = qkp.tile([128, NQ], bf16, tag="q", name=f"Qb{b}")
                Kb = qkp.tile([128, NK], bf16, tag="k", name=f"Kb{b}")
                VTb = vtp.tile([128, NCH * 128], bf16, tag="vt", name=f"VTb{b}")

                pq = psP.tile([128, 512], f32, tag="pp", name=f"pq{b}")
                nc.tensor.matmul(out=pq[:, :NQ], lhsT=w_q[:],
                                 rhs=Xq[:, b * NQ:(b + 1) * NQ],
                                 start=True, stop=True)
                nc.scalar.copy(out=Qb[:], in_=pq[:, :NQ])

                for half in range(2):
                    pk = psP.tile([128, 512], f32, tag="pp", name=f"pk{b}_{half}")
                    nc.tensor.matmul(out=pk[:, :NQ], lhsT=w_k[:],
                                     rhs=xkb[:, half * NQ:(half + 1) * NQ],
                                     start=True, stop=True)
                    if half:
                        nc.scalar.copy(out=Kb[:, NQ:], in_=pk[:, :NQ])
                    else:
                        nc.vector.tensor_copy(Kb[:, :NQ], pk[:, :NQ])

                pv1 = psP.tile([128, 512], f32, tag="pp", name=f"pv1{b}")
                for j in range(4):
                    nc.tensor.matmul(out=pv1[0:96, j * 128:(j + 1) * 128],
                                     lhsT=Xm[:, (b * NCH + j) * 96:
                                             (b * NCH + j + 1) * 96],
                                     rhs=w_v[:], start=True, stop=True)
                pv2 = psP.tile([128, 512], f32, tag="pp", name=f"pv2{b}")
                for j in range(4, 6):
                    nc.tensor.matmul(out=pv2[0:96, (j - 4) * 128:(j - 3) * 128],
                                     lhsT=Xm[:, (b * NCH + j) * 96:
                                             (b * NCH + j + 1) * 96],
                                     rhs=w_v[:], start=True, stop=True)
                nc.vector.tensor_copy(VTb[0:96, :512], pv1[0:96, :])
                nc.vector.tensor_copy(VTb[0:96, 512:768], pv2[0:96, :256])
                s["Q"], s["K"], s["VT"] = Qb, Kb, VTb

            def ph1(b, half):
                """Logits for heads {2*half, 2*half+1}: each head drains to
                its own psum bank (concurrent same-bank same-partition
                drains fault the HW)."""
                s = st[b]
                pl = psL.tile([128, 1024], mybir.dt.float32,
                              tag="plA" if half == 0 else "plB",
                              name=f"pl{b}_{half}")
                s[f"pl{half}"] = pl
                for j in range(NCH):
                    gl0, w, l0 = _chunk_geo(j)
                    for hh in range(2):
                        h = 2 * half + hh
                        dst = pl[0:96, hh * 512 + j * 84:
                                 hh * 512 + j * 84 + 84] \
                            .rearrange("p (r c) -> p r c", c=14)[:, :, l0:l0 + w]
                        lhsT = s["K"][32 * h:32 * h + 32,
                                      j * 96:(j + 1) * 96]
                        rhs = s["Q"][32 * h:32 * h + 32, :] \
                            .rearrange("p (r c) -> p r c", c=W2)[:, :, gl0:gl0 + w]
                        nc.tensor.matmul(out=dst, lhsT=lhsT, rhs=rhs,
                                         start=True, stop=True,
                                         tile_position=(32 * h, 0))

            def expmask(b, half):
                s = st[b]
                if half == 0:
                    s["att"] = attp.tile([128, 2016], mybir.dt.bfloat16,
                                         tag="att", name=f"att{b}")
                att = s["att"]
                src = s[f"pl{half}"][0:96, :].rearrange("p (k x) -> p k x",
                                                        k=2)[:, :, 0:504]
                dst = att[0:96, half * 1008:(half + 1) * 1008] \
                    .rearrange("p (k x) -> p k x", k=2)
                nc.scalar.activation(out=dst, in_=src,
                                     func=mybir.ActivationFunctionType.Exp,
                                     scale=SCALE)
                if _STAGES >= 2:
                    sl = slice(half * 1008, (half + 1) * 1008)
                    nc.vector.tensor_mul(out=att[0:96, sl],
                                         in0=att[0:96, sl],
                                         in1=WMt[0:96, 0:1008])

            def ph2(b, jlist):
                s = st[b]
                if "pO" not in s:
                    s["pO"] = psO.tile([128, 512], mybir.dt.float32, tag="po",
                                       name=f"pO{b}")
                    s["pS"] = psO.tile([128, 512], mybir.dt.float32, tag="ps",
                                       name=f"pS{b}")
                pO, pS = s["pO"], s["pS"]
                for j in jlist:
                    gl0, w, l0 = _chunk_geo(j)
                    for h in range(4):
                        attoff, _ = _unit_off(j, h)
                        rhs = s["att"][0:96, attoff:attoff + 84] \
                            .rearrange("p (r c) -> p r c", c=14)[:, :, l0:l0 + w]
                        dstO = pO[32 * h:32 * h + 32, :NQ] \
                            .rearrange("p (r c) -> p r c", c=W2)[:, :, gl0:gl0 + w]
                        nc.tensor.matmul(
                            out=dstO, lhsT=s["VT"][0:96, j * 128 + 32 * h:
                                                   j * 128 + 32 * h + 32],
                            rhs=rhs, start=(j == 0), stop=(j == 5),
                            tile_position=(0, 32 * h))
                        dstS = pS[32 * h:32 * h + 32, :NQ] \
                            .rearrange("p (r c) -> p r c", c=W2)[:, :, gl0:gl0 + w]
                        nc.tensor.matmul(
                            out=dstS,
                            lhsT=MKt[0:96, (b * NCH + j) * 32:
                                     (b * NCH + j) * 32 + 32],
                            rhs=rhs, start=(j == 0), stop=(j == 5),
                            tile_position=(0, 32 * h))

            def norm(b):
                # 1/pS as exp(-ln pS) on ACT (Ln/Exp share one act table);
                # DVE reciprocal is ~1.9us, this is ~0.85us off-DVE.
                s = st[b]
                lnS = nrm.tile([128, NQ], mybir.dt.float32, tag="lns",
                               name=f"lnS{b}")
                nc.scalar.activation(out=lnS[:], in_=s["pS"][:, :NQ],
                                     func=mybir.ActivationFunctionType.Ln)
                rcpS = nrm.tile([128, NQ], mybir.dt.float32, tag="rcp",
                                name=f"rcpS{b}")
                nc.scalar.activation(out=rcpS[:], in_=lnS[:],
                                     func=mybir.ActivationFunctionType.Exp,
                                     scale=-1.0)
                onrm = nrm.tile([128, NQ], mybir.dt.bfloat16, tag="on",
                                name=f"on{b}")
                nc.vector.tensor_mul(out=onrm[:], in0=s["pO"][:, :NQ],
                                     in1=rcpS[:])
                s["on"] = onrm

            def final(b):
                s = st[b]
                pf = psP.tile([128, 512], mybir.dt.float32, tag="pp",
                              name=f"pf{b}")
                nc.tensor.matmul(out=pf[:, :NQ], lhsT=w_p[:], rhs=s["on"][:],
                                 start=True, stop=True)
                osb = nrm.tile([128, NQ], mybir.dt.float32, tag="osb",
                               name=f"osb{b}")
                nc.scalar.copy(out=osb[:], in_=pf[:, :NQ])
                nc.gpsimd.dma_start(out=out_d[:, b * NQ:(b + 1) * NQ],
                                    in_=osb[:])
                st[b] = {}

            if not _PIPE:
                for b in range(_NRUN):
                    proj(b)
                    if _STAGES >= 1.3:
                        ph1(b, 0)
                        ph1(b, 1)
                    if _STAGES >= 1.6:
                        expmask(b, 0)
                        expmask(b, 1)
                    if _STAGES >= 3:
                        ph2(b, [0, 1, 2])
                        ph2(b, [3, 4, 5])
                    if _STAGES >= 4:
                        norm(b)
                    if _STAGES >= 5:
                        final(b)
            else:
                # 4-stage software pipeline per iteration `it`:
                #   proj(it+1) | ph1/exp/mask(it) | ph2/norm(it-1) | final(it-2)
                # so the PE never waits on the ACT exp / DVE mask chain.
                proj(0)
                for it in range(NBLK + 2):
                    if it < NBLK:
                        ph1(it, 0)
                        expmask(it, 0)
                    if it + 1 < NBLK:
                        proj(it + 1)
                    if it < NBLK:
                        ph1(it, 1)
                        expmask(it, 1)
                    if 0 <= it - 1 < NBLK:
                        ph2(it - 1, [0, 1, 2])
                        ph2(it - 1, [3, 4, 5])
                        norm(it - 1)
                    if 0 <= it - 2 < NBLK:
                        final(it - 2)

    _split_multi_waits(nc)
    return nc


def _split_multi_waits(nc):
    """This walrus build rejects >1 sem wait per instruction: move extra
    waits onto dedicated single-wait NoOps inserted just before."""
    import copy
    from concourse import mybir

    tmpl = nc.sync.nop(nofuse=True, hint="wsplit_template").ins
    bb0 = nc.cur_bb.bb
    bb0.instructions = [i for i in bb0.instructions if i.name != tmpl.name]
    tmpl = copy.deepcopy(tmpl)

    ctr = 0
    for f in nc.m.functions:
        for bb in f.blocks:
            insts = list(bb.instructions)
            new, changed = [], False
            for inst in insts:
                si = getattr(inst, "sync_info", None)
                waits = list(si.on_wait) if si is not None and si.on_wait else []
                if len(waits) > 1:
                    for w in waits[:-1]:
                        ctr += 1
                        nop = copy.deepcopy(tmpl)
                        nop.name = f"I-wsplit{ctr}"
                        nop.engine = inst.engine
                        nop.sync_info = mybir.SyncInfo(on_wait=[w], on_update=[])
                        new.append(nop)
                    si.on_wait = [waits[-1]]
                    changed = True
                new.append(inst)
            if changed:
                bb.instructions = new


def _host_prep(x, m):
    """Per-core inputs: xq [128, NBLK*NQ] row-major center rows; xk/xm
    [128, NBLK*NK] chunk-major (key p = (j, kr, kc')); mk [128, NBLK*6*32]."""
    import ml_dtypes
    bf = ml_dtypes.bfloat16
    # chunk-major permutation of a 576-key block
    perm = np.array([kr * W2 + 8 * j + kc
                     for j in range(NCH) for kr in range(KR)
                     for kc in range(8)], np.int64)
    xqs, xks, xms, mks = [], [], [], []
    mf = (m > 0).astype(np.float32)
    for k in range(CORES):
        r0 = 12 * k - 6
        xpad = np.zeros((B, C, 24, W), np.float32)
        mpad = np.zeros((B, 1, 24, W), np.float32)
        lo, hi = max(0, r0), min(H, r0 + 24)
        xpad[:, :, lo - r0:hi - r0] = x[:, :, lo:hi]
        mpad[:, :, lo - r0:hi - r0] = mf[:, :, lo:hi]
        xmp = xpad * mpad

        def coset(t, ch):
            v = t.reshape(B, ch, KR, 2, W2, 2).transpose(1, 0, 3, 5, 2, 4)
            return v.reshape(ch, NBLK, NK)

        xc = coset(xpad, C)                       # [C, NBLK, NK] row-major
        xq = xc[:, :, 144:144 + NQ].reshape(C, NBLK * NQ)
        xk = xc[:, :, perm].reshape(C, NBLK * NK)
        xm = coset(xmp, C)[:, :, perm].reshape(C, NBLK * NK)
        xqs.append(np.ascontiguousarray(xq).astype(bf))
        xks.append(np.ascontiguousarray(xk).astype(bf))
        xms.append(np.ascontiguousarray(xm).astype(bf))
        mc = coset(mpad, 1)[0][:, perm].reshape(NBLK, NCH, 96)
        mk = np.zeros((128, NBLK * NCH * 32), np.float32)
        vals = np.where(mc > 0, 1.0, EPS)         # [NBLK, NCH, 96]
        mk[0:96] = np.repeat(vals.reshape(NBLK * NCH, 96).T, 32, axis=1)
        mks.append(mk.astype(bf))
    return xqs, xks, xms, mks


def _host_wm():
    """[128, 1008] bf16: 0/1 window mask, unit layout [12 units][6 qr][14 lc],
    key partition p = kr*8 + kc'."""
    import ml_dtypes
    kr = np.arange(KR)[:, None, None, None]
    kc = np.arange(8)[None, :, None, None]
    qr = np.arange(CR)[None, None, :, None]
    lc = np.arange(14)[None, None, None, :]
    win = ((kr - qr >= 0) & (kr - qr <= 6) & (lc >= kc) & (lc <= kc + 6))
    unit = win.reshape(96, 84).astype(np.float32)
    wm = np.zeros((128, 1008), np.float32)
    wm[0:96] = np.tile(unit, (1, 12))
    return wm.astype(ml_dtypes.bfloat16)


def _make_in_maps(x, m, Wq, Wk, Wv, Wp):
    import ml_dtypes
    bf = ml_dtypes.bfloat16
    xqs, xks, xms, mks = _host_prep(np.asarray(x, np.float32),
                                    np.asarray(m, np.int32))
    base = {
        "wm": _host_wm(),
        "wq": np.ascontiguousarray(np.asarray(Wq, np.float32).T).astype(bf),
        "wk": np.ascontiguousarray(np.asarray(Wk, np.float32).T).astype(bf),
        "wv": np.ascontiguousarray(np.asarray(Wv, np.float32).T).astype(bf),
        "wp": np.ascontiguousarray(np.asarray(Wp, np.float32).T).astype(bf),
    }
    return [{**base, "xq": xqs[k], "xk": xks[k], "xm": xms[k], "mk": mks[k]}
            for k in range(CORES)]


def kernel(x, m, Wq, Wk, Wv, Wp):
    global _prog
    from concourse.bass_utils import run_bass_kernel_spmd

    if _prog is None:
        _prog = _build_program()

    in_maps = _make_in_maps(x, m, Wq, Wk, Wv, Wp)
    res = run_bass_kernel_spmd(_prog, in_maps, list(range(CORES)))

    full = np.zeros((B, C, H, W), np.float32)
    for k in range(CORES):
        oc = res.results[k]["out"].reshape(C, B, 2, 2, CR, W2)
        o = oc.transpose(1, 0, 4, 2, 5, 3).reshape(B, C, 12, 96)
        full[:, :, 12 * k:12 * k + 12, :] = o
    return full


# revision 38
# speedup vs baseline: 3.0274x; 1.0777x over previous
"""Dilated (dil=2) 7x7 window self-attention, 4 heads x 32 dim, on 8 trn2 cores.

v2: spatial sharding over image rows (12 rows/core, 6-row halo), 4 cosets
(row/col parity) x 2 batches = 8 independent blocks per core.  Within a
block the coset grid is 6 query rows x 48 cols (NQ=288) attending over
12 key rows x 48 cols (NK=576) with a dense 7x7 window (|dr|,|dc| <= 3
in coset space; local key row kr attends query rows qr in [kr-6, kr]).

All matmuls bf16 (tolerance 2e-2 gives plenty of slack):
  - keys split into 6 column-chunks of 8 cols (96 keys = 12r x 8c each);
    queries touched by chunk j = 6 rows x 14 cols (global cols 8j-3..
    8j+10, clipped) -> logits unit [96 keys, 6x14=84] per (chunk, head).
  - phase 1: one matmul per (chunk, head): lhsT = K chunk [32, 96],
    rhs = Q window [32, 6, w] -> psum unit; 4 heads packed via
    tile_position rows.  24 units = 4 psum banks (6 units x 84 per bank).
  - exp: one ACT instruction per 2-bank half (12 units), no bias, no max
    subtraction (logits are tiny); writes bf16 attnT.
  - window mask: one bf16 multiply per half with a precomputed 0/1 mask
    (same for every unit).
  - key masking: V is projected from host-premultiplied x*m, so masked
    and padding keys contribute 0 to the numerator; the denominator is a
    matmul with lhsT = per-(block,chunk) key validity (eps for invalid)
    replicated x32, so invalid keys contribute ~eps.
  - phase 2: per chunk, 4 pO + 4 pS matmuls (col-tiled by head),
    accumulated across chunks into overlapping [32h, 6, w] psum windows.
  - normalize: reciprocal_approx_fast(pS) * pO -> bf16, then the 1x1
    output projection and a psum->sbuf fp32 copy + DMA out.

Blocks are software-pipelined: projections of block b+1 are emitted
between phase 1 and phase 2 of block b so the PE never waits on the
ACT/DVE exp/mask chain.
"""

import numpy as np

HEADS, D, WIN, DIL = 4, 32, 7, 2
B, C, H, W = 2, 128, 96, 96
CORES = 8
CR, KR, W2 = 6, 12, 48            # coset query rows / key rows (halo) / cols
NQ, NK = CR * W2, KR * W2         # 288, 576
NBLK = B * 4                      # (batch, coset) blocks per core
NCH = 6                           # key column chunks of 8
SCALE = float(1.0 / np.sqrt(D))
EPS = 1e-5                        # denominator weight for invalid keys
_PIPE = True                      # software-pipeline blocks
_NRUN = NBLK                      # blocks to emit in no-pipe debug mode
_STAGES = 5                       # no-pipe debug: how many stages to emit

_prog = None


def _chunk_geo(j):
    """(gl0, w, l0): global q-col start, width, offset in 14-col frame."""
    gl0 = max(0, 8 * j - 3)
    gl1 = min(W2 - 1, 8 * j + 10)
    return gl0, gl1 - gl0 + 1, gl0 - (8 * j - 3)


def _unit_off(j, h):
    """attnT / psum offsets of unit (chunk j, head h).  Bank h holds head
    h's six 84-wide units — concurrent head-tiles must drain to DISTINCT
    psum banks (same-bank same-partition concurrent drains fault the HW)."""
    att = h * 504 + j * 84
    pl = h * 512 + j * 84
    return att, pl


def _build_program():
    import concourse.bass as bass
    import concourse.tile as tile
    from concourse import mybir

    nc = bass.Bass("TRN2", target_bir_lowering=False, debug=False,
                   num_devices=CORES)
    f32 = mybir.dt.float32
    bf16 = mybir.dt.bfloat16

    xq_d = nc.dram_tensor("xq", [128, NBLK * NQ], bf16, kind="ExternalInput").ap()
    xk_d = nc.dram_tensor("xk", [128, NBLK * NK], bf16, kind="ExternalInput").ap()
    xm_d = nc.dram_tensor("xm", [128, NBLK * NK], bf16, kind="ExternalInput").ap()
    wm_d = nc.dram_tensor("wm", [128, 1008], bf16, kind="ExternalInput").ap()
    mk_d = nc.dram_tensor("mk", [128, NBLK * NCH * 32], bf16,
                          kind="ExternalInput").ap()
    wq_d = nc.dram_tensor("wq", [128, 128], bf16, kind="ExternalInput").ap()
    wk_d = nc.dram_tensor("wk", [128, 128], bf16, kind="ExternalInput").ap()
    wv_d = nc.dram_tensor("wv", [128, 128], bf16, kind="ExternalInput").ap()
    wp_d = nc.dram_tensor("wp", [128, 128], bf16, kind="ExternalInput").ap()
    out_d = nc.dram_tensor("out", [128, NBLK * NQ], f32,
                           kind="ExternalOutput").ap()

    with tile.TileContext(nc) as tc:
        with tc.tile_pool(name="cst", bufs=1) as cst, \
             tc.tile_pool(name="att", bufs=2) as attp, \
             tc.tile_pool(name="nrm", bufs=2) as nrm, \
             tc.tile_pool(name="psL", bufs=1, space="PSUM") as psL, \
             tc.tile_pool(name="psO", bufs=1, space="PSUM") as psO, \
             tc.tile_pool(name="psP", bufs=2, space="PSUM") as psP:

            # DMA issue cost is ~600ns per dma_start on an engine queue —
            # spread issuance across idle engines, earliest-needed first.
            w_q = cst.tile([128, 128], bf16)
            nc.scalar.dma_start(out=w_q[:], in_=wq_d[:])
            w_k = cst.tile([128, 128], bf16)
            nc.scalar.dma_start(out=w_k[:], in_=wk_d[:])
            w_v = cst.tile([128, 128], bf16)
            nc.scalar.dma_start(out=w_v[:], in_=wv_d[:])
            w_p = cst.tile([128, 128], bf16)
            nc.scalar.dma_start(out=w_p[:], in_=wp_d[:])
            WMt = cst.tile([128, 1008], bf16)
            nc.gpsimd.dma_start(out=WMt[:], in_=wm_d[:])
            MKt = cst.tile([128, NBLK * NCH * 32], bf16)
            nc.gpsimd.dma_start(out=MKt[:], in_=mk_d[:])

            Xq = cst.tile([128, NBLK * NQ], bf16)
            Xk = cst.tile([128, NBLK * NK], bf16)
            Xm = cst.tile([128, NBLK * NK], bf16)
            for q in range(2):
                sl = slice(q * NBLK * NK // 2, (q + 1) * NBLK * NK // 2)
                nc.sync.dma_start(out=Xk[:, sl], in_=xk_d[:, sl])
                nc.gpsimd.dma_start(out=Xm[:, sl], in_=xm_d[:, sl])
                s2 = slice(q * NBLK * NQ // 2, (q + 1) * NBLK * NQ // 2)
                nc.sync.dma_start(out=Xq[:, s2], in_=xq_d[:, s2])

            # zero the psL ring slots once so exp of never-written lanes
            # stays bounded
            plz0 = psL.tile([128, 1024], f32, tag="plA")
            nc.vector.memset(plz0[:], 0.0)
            plz1 = psL.tile([128, 1024], f32, tag="plB")
            nc.vector.memset(plz1[:], 0.0)

            # per-block state carried between pipeline stages
            st = [dict() for _ in range(NBLK)]

            Qall = cst.tile([128, NBLK * NQ], bf16)
            Kall = cst.tile([128, NBLK * NK], bf16)
            VTall = cst.tile([128, NBLK * NCH * 128], bf16)

            def proj_all():
                """All projections up front: 512-wide matmuls, copies
                alternating between ACT and DVE."""
                eng = [nc.scalar, nc.vector]
                ncopy = 0

                def emit(pt, dst_ap, n, par=128):
                    nonlocal ncopy
                    e = eng[ncopy % 2]
                    if e is nc.scalar:
                        e.copy(out=dst_ap, in_=pt[0:par, :n])
                    else:
                        e.tensor_copy(dst_ap, pt[0:par, :n])
                    ncopy += 1

                def kstep(i):
                    n = min(512, NBLK * NK - i * 512)
                    pk = psP.tile([128, 512], f32, tag="pp", name=f"pk{i}")
                    nc.tensor.matmul(out=pk[:, :n], lhsT=w_k[:],
                                     rhs=Xk[:, i * 512:i * 512 + n],
                                     start=True, stop=True)
                    emit(pk, Kall[:, i * 512:i * 512 + n], n)

                def qstep(i):
                    n = min(512, NBLK * NQ - i * 512)
                    pq = psP.tile([128, 512], f32, tag="pp", name=f"pq{i}")
                    nc.tensor.matmul(out=pq[:, :n], lhsT=w_q[:],
                                     rhs=Xq[:, i * 512:i * 512 + n],
                                     start=True, stop=True)
                    emit(pq, Qall[:, i * 512:i * 512 + n], n)

                def vstep(g):
                    pv = psP.tile([128, 512], f32, tag="pp", name=f"pv{g}")
                    for c in range(4):
                        u = g * 4 + c
                        nc.tensor.matmul(out=pv[0:96, c * 128:(c + 1) * 128],
                                         lhsT=Xm[:, u * 96:(u + 1) * 96],
                                         rhs=w_v[:], start=True, stop=True)
                    emit(pv, VTall[0:96, g * 512:(g + 1) * 512], 512, par=96)

                # block-0-first interleave so the attention loop starts early
                sched = ([("k", 0), ("k", 1), ("q", 0), ("v", 0), ("v", 1),
                          ("q", 1)] +
                         [x for i in range(2, 9)
                          for x in [("k", i), ("v", i), ("v", i + 5)]
                          ] + [("q", i) for i in range(2, 5)])
                seen = set()
                for kind, i in sched:
                    if (kind, i) in seen or (kind == "v" and i > 11):
                        continue
                    seen.add((kind, i))
                    {"k": kstep, "q": qstep, "v": vstep}[kind](i)
                for i in range(12):
                    if ("v", i) not in seen:
                        vstep(i)

            def ph1(b, half):
                """Logits for heads {2*half, 2*half+1}: each head drains to
                its own psum bank (concurrent same-bank same-partition
                drains fault the HW)."""
                s = st[b]
                pl = psL.tile([128, 1024], mybir.dt.float32,
                              tag="plA" if half == 0 else "plB",
                              name=f"pl{b}_{half}")
                s[f"pl{half}"] = pl
                for j in range(NCH):
                    gl0, w, l0 = _chunk_geo(j)
                    for hh in range(2):
                        h = 2 * half + hh
                        dst = pl[0:96, hh * 512 + j * 84:
                                 hh * 512 + j * 84 + 84] \
                            .rearrange("p (r c) -> p r c", c=14)[:, :, l0:l0 + w]
                        lhsT = Kall[32 * h:32 * h + 32,
                                    b * NK + j * 96:b * NK + (j + 1) * 96]
                        rhs = Qall[32 * h:32 * h + 32,
                                   b * NQ:(b + 1) * NQ] \
                            .rearrange("p (r c) -> p r c", c=W2)[:, :, gl0:gl0 + w]
                        nc.tensor.matmul(out=dst, lhsT=lhsT, rhs=rhs,
                                         start=True, stop=True,
                                         tile_position=(32 * h, 0))

            def expmask(b, half):
                s = st[b]
                if half == 0:
                    s["att"] = attp.tile([128, 2016], mybir.dt.bfloat16,
                                         tag="att", name=f"att{b}")
                att = s["att"]
                src = s[f"pl{half}"][0:96, :].rearrange("p (k x) -> p k x",
                                                        k=2)[:, :, 0:504]
                dst = att[0:96, half * 1008:(half + 1) * 1008] \
                    .rearrange("p (k x) -> p k x", k=2)
                nc.scalar.activation(out=dst, in_=src,
                                     func=mybir.ActivationFunctionType.Exp,
                                     scale=SCALE)
                if _STAGES >= 2:
                    sl = slice(half * 1008, (half + 1) * 1008)
                    nc.vector.tensor_mul(out=att[0:96, sl],
                                         in0=att[0:96, sl],
                                         in1=WMt[0:96, 0:1008])

            def ph2(b, jlist):
                s = st[b]
                if "pO" not in s:
                    s["pO"] = psO.tile([128, 512], mybir.dt.float32, tag="po",
                                       name=f"pO{b}")
                    s["pS"] = psO.tile([128, 512], mybir.dt.float32, tag="ps",
                                       name=f"pS{b}")
                pO, pS = s["pO"], s["pS"]
                for j in jlist:
                    gl0, w, l0 = _chunk_geo(j)
                    for h in range(4):
                        attoff, _ = _unit_off(j, h)
                        rhs = s["att"][0:96, attoff:attoff + 84] \
                            .rearrange("p (r c) -> p r c", c=14)[:, :, l0:l0 + w]
                        dstO = pO[32 * h:32 * h + 32, :NQ] \
                            .rearrange("p (r c) -> p r c", c=W2)[:, :, gl0:gl0 + w]
                        nc.tensor.matmul(
                            out=dstO,
                            lhsT=VTall[0:96, b * 768 + j * 128 + 32 * h:
                                       b * 768 + j * 128 + 32 * h + 32],
                            rhs=rhs, start=(j == 0), stop=(j == 5),
                            tile_position=(0, 32 * h))
                        dstS = pS[32 * h:32 * h + 32, :NQ] \
                            .rearrange("p (r c) -> p r c", c=W2)[:, :, gl0:gl0 + w]
                        nc.tensor.matmul(
                            out=dstS,
                            lhsT=MKt[0:96, (b * NCH + j) * 32:
                                     (b * NCH + j) * 32 + 32],
                            rhs=rhs, start=(j == 0), stop=(j == 5),
                            tile_position=(0, 32 * h))

            def norm(b):
                # 1/pS as exp(-ln pS) on ACT (Ln/Exp share one act table);
                # DVE reciprocal is ~1.9us, this is ~0.85us off-DVE.
                s = st[b]
                lnS = nrm.tile([128, NQ], mybir.dt.float32, tag="lns",
                               name=f"lnS{b}")
                nc.scalar.activation(out=lnS[:], in_=s["pS"][:, :NQ],
                                     func=mybir.ActivationFunctionType.Ln)
                rcpS = nrm.tile([128, NQ], mybir.dt.float32, tag="rcp",
                                name=f"rcpS{b}")
                nc.scalar.activation(out=rcpS[:], in_=lnS[:],
                                     func=mybir.ActivationFunctionType.Exp,
                                     scale=-1.0)
                onrm = nrm.tile([128, NQ], mybir.dt.bfloat16, tag="on",
                                name=f"on{b}")
                nc.vector.tensor_mul(out=onrm[:], in0=s["pO"][:, :NQ],
                                     in1=rcpS[:])
                s["on"] = onrm

            def final(b):
                s = st[b]
                pf = psP.tile([128, 512], mybir.dt.float32, tag="pp",
                              name=f"pf{b}")
                nc.tensor.matmul(out=pf[:, :NQ], lhsT=w_p[:], rhs=s["on"][:],
                                 start=True, stop=True)
                osb = nrm.tile([128, NQ], mybir.dt.float32, tag="osb",
                               name=f"osb{b}")
                nc.vector.tensor_copy(osb[:], pf[:, :NQ])
                nc.sync.dma_start(out=out_d[:, b * NQ:(b + 1) * NQ],
                                  in_=osb[:])
                st[b] = {}

            proj_all()
            if not _PIPE:
                for b in range(_NRUN):
                    if _STAGES >= 1.3:
                        ph1(b, 0)
                        ph1(b, 1)
                    if _STAGES >= 1.6:
                        expmask(b, 0)
                        expmask(b, 1)
                    if _STAGES >= 3:
                        ph2(b, [0, 1, 2])
                        ph2(b, [3, 4, 5])
                    if _STAGES >= 4:
                        norm(b)
                    if _STAGES >= 5:
                        final(b)
            else:
                # 3-stage software pipeline per iteration `it`:
                #   ph1/exp/mask(it) | ph2/norm(it-1) | final(it-2)
                for it in range(NBLK + 2):
                    if it < NBLK:
                        ph1(it, 0)
                        expmask(it, 0)
                        ph1(it, 1)
                        expmask(it, 1)
                    if 0 <= it - 1 < NBLK:
                        ph2(it - 1, [0, 1, 2])
                        ph2(it - 1, [3, 4, 5])
                        norm(it - 1)
                    if 0 <= it - 2 < NBLK:
                        final(it - 2)

    _split_multi_waits(nc)
    return nc


def _split_multi_waits(nc):
    """This walrus build rejects >1 sem wait per instruction: move extra
    waits onto dedicated single-wait NoOps inserted just before."""
    import copy
    from concourse import mybir

    tmpl = nc.sync.nop(nofuse=True, hint="wsplit_template").ins
    bb0 = nc.cur_bb.bb
    bb0.instructions = [i for i in bb0.instructions if i.name != tmpl.name]
    tmpl = copy.deepcopy(tmpl)

    ctr = 0
    for f in nc.m.functions:
        for bb in f.blocks:
            insts = list(bb.instructions)
            new, changed = [], False
            for inst in insts:
                si = getattr(inst, "sync_info", None)
                waits = list(si.on_wait) if si is not None and si.on_wait else []
                if len(waits) > 1:
                    for w in waits[:-1]:
                        ctr += 1
                        nop = copy.deepcopy(tmpl)
                        nop.name = f"I-wsplit{ctr}"
                        nop.engine = inst.engine
                        nop.sync_info = mybir.SyncInfo(on_wait=[w], on_update=[])
                        new.append(nop)
                    si.on_wait = [waits[-1]]
                    changed = True
                new.append(inst)
            if changed:
                bb.instructions = new


def _host_prep(x, m):
    """Per-core inputs: xq [128, NBLK*NQ] row-major center rows; xk/xm
    [128, NBLK*NK] chunk-major (key p = (j, kr, kc')); mk [128, NBLK*6*32]."""
    import ml_dtypes
    bf = ml_dtypes.bfloat16
    # chunk-major permutation of a 576-key block
    perm = np.array([kr * W2 + 8 * j + kc
                     for j in range(NCH) for kr in range(KR)
                     for kc in range(8)], np.int64)
    xqs, xks, xms, mks = [], [], [], []
    mf = (m > 0).astype(np.float32)
    for k in range(CORES):
        r0 = 12 * k - 6
        xpad = np.zeros((B, C, 24, W), np.float32)
        mpad = np.zeros((B, 1, 24, W), np.float32)
        lo, hi = max(0, r0), min(H, r0 + 24)
        xpad[:, :, lo - r0:hi - r0] = x[:, :, lo:hi]
        mpad[:, :, lo - r0:hi - r0] = mf[:, :, lo:hi]
        xmp = xpad * mpad

        def coset(t, ch):
            v = t.reshape(B, ch, KR, 2, W2, 2).transpose(1, 0, 3, 5, 2, 4)
            return v.reshape(ch, NBLK, NK)

        xc = coset(xpad, C)                       # [C, NBLK, NK] row-major
        xq = xc[:, :, 144:144 + NQ].reshape(C, NBLK * NQ)
        xk = xc[:, :, perm].reshape(C, NBLK * NK)
        xm = coset(xmp, C)[:, :, perm].reshape(C, NBLK * NK)
        xqs.append(np.ascontiguousarray(xq).astype(bf))
        xks.append(np.ascontiguousarray(xk).astype(bf))
        xms.append(np.ascontiguousarray(xm).astype(bf))
        mc = coset(mpad, 1)[0][:, perm].reshape(NBLK, NCH, 96)
        mk = np.zeros((128, NBLK * NCH * 32), np.float32)
        vals = np.where(mc > 0, 1.0, EPS)         # [NBLK, NCH, 96]
        mk[0:96] = np.repeat(vals.reshape(NBLK * NCH, 96).T, 32, axis=1)
        mks.append(mk.astype(bf))
    return xqs, xks, xms, mks


def _host_wm():
    """[128, 1008] bf16: 0/1 window mask, unit layout [12 units][6 qr][14 lc],
    key partition p = kr*8 + kc'."""
    import ml_dtypes
    kr = np.arange(KR)[:, None, None, None]
    kc = np.arange(8)[None, :, None, None]
    qr = np.arange(CR)[None, None, :, None]
    lc = np.arange(14)[None, None, None, :]
    win = ((kr - qr >= 0) & (kr - qr <= 6) & (lc >= kc) & (lc <= kc + 6))
    unit = win.reshape(96, 84).astype(np.float32)
    wm = np.zeros((128, 1008), np.float32)
    wm[0:96] = np.tile(unit, (1, 12))
    return wm.astype(ml_dtypes.bfloat16)


def _make_in_maps(x, m, Wq, Wk, Wv, Wp):
    import ml_dtypes
    bf = ml_dtypes.bfloat16
    xqs, xks, xms, mks = _host_prep(np.asarray(x, np.float32),
                                    np.asarray(m, np.int32))
    base = {
        "wm": _host_wm(),
        "wq": np.ascontiguousarray(np.asarray(Wq, np.float32).T).astype(bf),
        "wk": np.ascontiguousarray(np.asarray(Wk, np.float32).T).astype(bf),
        "wv": np.ascontiguousarray(np.asarray(Wv, np.float32).T).astype(bf),
        "wp": np.ascontiguousarray(np.asarray(Wp, np.float32).T).astype(bf),
    }
    return [{**base, "xq": xqs[k], "xk": xks[k], "xm": xms[k], "mk": mks[k]}
            for k in range(CORES)]


def kernel(x, m, Wq, Wk, Wv, Wp):
    global _prog
    from concourse.bass_utils import run_bass_kernel_spmd

    if _prog is None:
        _prog = _build_program()

    in_maps = _make_in_maps(x, m, Wq, Wk, Wv, Wp)
    res = run_bass_kernel_spmd(_prog, in_maps, list(range(CORES)))

    full = np.zeros((B, C, H, W), np.float32)
    for k in range(CORES):
        oc = res.results[k]["out"].reshape(C, B, 2, 2, CR, W2)
        o = oc.transpose(1, 0, 4, 2, 5, 3).reshape(B, C, 12, 96)
        full[:, :, 12 * k:12 * k + 12, :] = o
    return full


# revision 39
# speedup vs baseline: 3.2450x; 1.0719x over previous
"""Dilated (dil=2) 7x7 window self-attention, 4 heads x 32 dim, on 8 trn2 cores.

v2: spatial sharding over image rows (12 rows/core, 6-row halo), 4 cosets
(row/col parity) x 2 batches = 8 independent blocks per core.  Within a
block the coset grid is 6 query rows x 48 cols (NQ=288) attending over
12 key rows x 48 cols (NK=576) with a dense 7x7 window (|dr|,|dc| <= 3
in coset space; local key row kr attends query rows qr in [kr-6, kr]).

All matmuls bf16 (tolerance 2e-2 gives plenty of slack):
  - keys split into 6 column-chunks of 8 cols (96 keys = 12r x 8c each);
    queries touched by chunk j = 6 rows x 14 cols (global cols 8j-3..
    8j+10, clipped) -> logits unit [96 keys, 6x14=84] per (chunk, head).
  - phase 1: one matmul per (chunk, head): lhsT = K chunk [32, 96],
    rhs = Q window [32, 6, w] -> psum unit; 4 heads packed via
    tile_position rows.  24 units = 4 psum banks (6 units x 84 per bank).
  - exp: one ACT instruction per 2-bank half (12 units), no bias, no max
    subtraction (logits are tiny); writes bf16 attnT.
  - window mask: one bf16 multiply per half with a precomputed 0/1 mask
    (same for every unit).
  - key masking: V is projected from host-premultiplied x*m, so masked
    and padding keys contribute 0 to the numerator; the denominator is a
    matmul with lhsT = per-(block,chunk) key validity (eps for invalid)
    replicated x32, so invalid keys contribute ~eps.
  - phase 2: per chunk, 4 pO + 4 pS matmuls (col-tiled by head),
    accumulated across chunks into overlapping [32h, 6, w] psum windows.
  - normalize: reciprocal_approx_fast(pS) * pO -> bf16, then the 1x1
    output projection and a psum->sbuf fp32 copy + DMA out.

Blocks are software-pipelined: projections of block b+1 are emitted
between phase 1 and phase 2 of block b so the PE never waits on the
ACT/DVE exp/mask chain.
"""

import numpy as np

HEADS, D, WIN, DIL = 4, 32, 7, 2
B, C, H, W = 2, 128, 96, 96
CORES = 8
CR, KR, W2 = 6, 12, 48            # coset query rows / key rows (halo) / cols
NQ, NK = CR * W2, KR * W2         # 288, 576
NBLK = B * 4                      # (batch, coset) blocks per core
NCH = 6                           # key column chunks of 8
SCALE = float(1.0 / np.sqrt(D))
EPS = 1e-5                        # denominator weight for invalid keys
_PIPE = True                      # software-pipeline blocks
_NRUN = NBLK                      # blocks to emit in no-pipe debug mode
_STAGES = 5                       # no-pipe debug: how many stages to emit

_prog = None


def _chunk_geo(j):
    """(gl0, w, l0): global q-col start, width, offset in 14-col frame."""
    gl0 = max(0, 8 * j - 3)
    gl1 = min(W2 - 1, 8 * j + 10)
    return gl0, gl1 - gl0 + 1, gl0 - (8 * j - 3)


def _unit_off(j, h):
    """attnT / psum offsets of unit (chunk j, head h).  Bank h holds head
    h's six 84-wide units — concurrent head-tiles must drain to DISTINCT
    psum banks (same-bank same-partition concurrent drains fault the HW)."""
    att = h * 504 + j * 84
    pl = h * 512 + j * 84
    return att, pl


def _build_program():
    import concourse.bass as bass
    import concourse.tile as tile
    from concourse import mybir

    nc = bass.Bass("TRN2", target_bir_lowering=False, debug=False,
                   num_devices=CORES)
    f32 = mybir.dt.float32
    bf16 = mybir.dt.bfloat16

    xq_d = nc.dram_tensor("xq", [128, NBLK * NQ], bf16, kind="ExternalInput").ap()
    xk_d = nc.dram_tensor("xk", [128, NBLK * NK], bf16, kind="ExternalInput").ap()
    xm_d = nc.dram_tensor("xm", [128, NBLK * NK], bf16, kind="ExternalInput").ap()
    wm_d = nc.dram_tensor("wm", [128, 1008], bf16, kind="ExternalInput").ap()
    mk_d = nc.dram_tensor("mk", [128, NBLK * NCH * 32], bf16,
                          kind="ExternalInput").ap()
    wq_d = nc.dram_tensor("wq", [128, 128], bf16, kind="ExternalInput").ap()
    wk_d = nc.dram_tensor("wk", [128, 128], bf16, kind="ExternalInput").ap()
    wv_d = nc.dram_tensor("wv", [128, 128], bf16, kind="ExternalInput").ap()
    wp_d = nc.dram_tensor("wp", [128, 128], bf16, kind="ExternalInput").ap()
    out_d = nc.dram_tensor("out", [128, NBLK * NQ], f32,
                           kind="ExternalOutput").ap()

    with tile.TileContext(nc) as tc:
        with tc.tile_pool(name="cst", bufs=1) as cst, \
             tc.tile_pool(name="att", bufs=2) as attp, \
             tc.tile_pool(name="nrm", bufs=2) as nrm, \
             tc.tile_pool(name="psL", bufs=1, space="PSUM") as psL, \
             tc.tile_pool(name="psO", bufs=1, space="PSUM") as psO, \
             tc.tile_pool(name="psP", bufs=2, space="PSUM") as psP:

            # DMA issue cost is ~600ns per dma_start on an engine queue —
            # spread issuance across idle engines, earliest-needed first.
            w_q = cst.tile([128, 128], bf16)
            nc.scalar.dma_start(out=w_q[:], in_=wq_d[:])
            w_k = cst.tile([128, 128], bf16)
            nc.scalar.dma_start(out=w_k[:], in_=wk_d[:])
            w_v = cst.tile([128, 128], bf16)
            nc.scalar.dma_start(out=w_v[:], in_=wv_d[:])
            w_p = cst.tile([128, 128], bf16)
            nc.scalar.dma_start(out=w_p[:], in_=wp_d[:])
            WMt = cst.tile([128, 1008], bf16)
            nc.gpsimd.dma_start(out=WMt[:], in_=wm_d[:])
            MKt = cst.tile([128, NBLK * NCH * 32], bf16)
            nc.gpsimd.dma_start(out=MKt[:], in_=mk_d[:])

            Xq = cst.tile([128, NBLK * NQ], bf16)
            Xk = cst.tile([128, NBLK * NK], bf16)
            Xm = cst.tile([128, NBLK * NK], bf16)
            for q in range(2):
                sl = slice(q * NBLK * NK // 2, (q + 1) * NBLK * NK // 2)
                nc.sync.dma_start(out=Xk[:, sl], in_=xk_d[:, sl])
                nc.gpsimd.dma_start(out=Xm[:, sl], in_=xm_d[:, sl])
                s2 = slice(q * NBLK * NQ // 2, (q + 1) * NBLK * NQ // 2)
                nc.sync.dma_start(out=Xq[:, s2], in_=xq_d[:, s2])

            # zero the psL ring slots once so exp of never-written lanes
            # stays bounded
            plz0 = psL.tile([128, 1024], f32, tag="plA")
            nc.vector.memset(plz0[:], 0.0)
            plz1 = psL.tile([128, 1024], f32, tag="plB")
            nc.vector.memset(plz1[:], 0.0)

            # per-block state carried between pipeline stages
            st = [dict() for _ in range(NBLK)]

            Qall = cst.tile([128, NBLK * NQ], bf16)
            Kall = cst.tile([128, NBLK * NK], bf16)
            VTall = cst.tile([128, NBLK * NCH * 128], bf16)

            def proj_all():
                """All projections up front: 512-wide matmuls, copies
                alternating between ACT and DVE."""
                eng = [nc.scalar, nc.vector]
                ncopy = 0
                # During the projection phase the attention psum banks are
                # idle: rotate over 6 slots so matmuls never wait on copies.
                slots = [(psP, "pp"), (psP, "pp"), (psL, "plA"),
                         (psL, "plB"), (psO, "po"), (psO, "ps")]
                nalloc = [0]

                def ptile(name):
                    pool, tag = slots[nalloc[0] % 6]
                    nalloc[0] += 1
                    return pool.tile([128, 512], f32, tag=tag, name=name)

                def emit(pt, dst_ap, n, par=128):
                    nonlocal ncopy
                    e = eng[ncopy % 2]
                    if e is nc.scalar:
                        e.copy(out=dst_ap, in_=pt[0:par, :n])
                    else:
                        e.tensor_copy(dst_ap, pt[0:par, :n])
                    ncopy += 1

                def kstep(i):
                    n = min(512, NBLK * NK - i * 512)
                    pk = ptile(f"pk{i}")
                    nc.tensor.matmul(out=pk[:, :n], lhsT=w_k[:],
                                     rhs=Xk[:, i * 512:i * 512 + n],
                                     start=True, stop=True)
                    emit(pk, Kall[:, i * 512:i * 512 + n], n)

                def qstep(i):
                    n = min(512, NBLK * NQ - i * 512)
                    pq = ptile(f"pq{i}")
                    nc.tensor.matmul(out=pq[:, :n], lhsT=w_q[:],
                                     rhs=Xq[:, i * 512:i * 512 + n],
                                     start=True, stop=True)
                    emit(pq, Qall[:, i * 512:i * 512 + n], n)

                def vstep(g):
                    pv = ptile(f"pv{g}")
                    for c in range(4):
                        u = g * 4 + c
                        nc.tensor.matmul(out=pv[0:96, c * 128:(c + 1) * 128],
                                         lhsT=Xm[:, u * 96:(u + 1) * 96],
                                         rhs=w_v[:], start=True, stop=True)
                    emit(pv, VTall[0:96, g * 512:(g + 1) * 512], 512, par=96)

                # block-0-first interleave so the attention loop starts early
                sched = ([("k", 0), ("k", 1), ("q", 0), ("v", 0), ("v", 1),
                          ("q", 1)] +
                         [x for i in range(2, 9)
                          for x in [("k", i), ("v", i), ("v", i + 5)]
                          ] + [("q", i) for i in range(2, 5)])
                seen = set()
                for kind, i in sched:
                    if (kind, i) in seen or (kind == "v" and i > 11):
                        continue
                    seen.add((kind, i))
                    {"k": kstep, "q": qstep, "v": vstep}[kind](i)
                for i in range(12):
                    if ("v", i) not in seen:
                        vstep(i)

            def ph1(b, half):
                """Logits for heads {2*half, 2*half+1}: each head drains to
                its own psum bank (concurrent same-bank same-partition
                drains fault the HW)."""
                s = st[b]
                pl = psL.tile([128, 1024], mybir.dt.float32,
                              tag="plA" if half == 0 else "plB",
                              name=f"pl{b}_{half}")
                s[f"pl{half}"] = pl
                for j in range(NCH):
                    gl0, w, l0 = _chunk_geo(j)
                    for hh in range(2):
                        h = 2 * half + hh
                        dst = pl[0:96, hh * 512 + j * 84:
                                 hh * 512 + j * 84 + 84] \
                            .rearrange("p (r c) -> p r c", c=14)[:, :, l0:l0 + w]
                        lhsT = Kall[32 * h:32 * h + 32,
                                    b * NK + j * 96:b * NK + (j + 1) * 96]
                        rhs = Qall[32 * h:32 * h + 32,
                                   b * NQ:(b + 1) * NQ] \
                            .rearrange("p (r c) -> p r c", c=W2)[:, :, gl0:gl0 + w]
                        nc.tensor.matmul(out=dst, lhsT=lhsT, rhs=rhs,
                                         start=True, stop=True,
                                         tile_position=(32 * h, 0))

            def expmask(b, half):
                s = st[b]
                if half == 0:
                    s["att"] = attp.tile([128, 2016], mybir.dt.bfloat16,
                                         tag="att", name=f"att{b}")
                att = s["att"]
                src = s[f"pl{half}"][0:96, :].rearrange("p (k x) -> p k x",
                                                        k=2)[:, :, 0:504]
                dst = att[0:96, half * 1008:(half + 1) * 1008] \
                    .rearrange("p (k x) -> p k x", k=2)
                nc.scalar.activation(out=dst, in_=src,
                                     func=mybir.ActivationFunctionType.Exp,
                                     scale=SCALE)
                if _STAGES >= 2:
                    sl = slice(half * 1008, (half + 1) * 1008)
                    nc.vector.tensor_mul(out=att[0:96, sl],
                                         in0=att[0:96, sl],
                                         in1=WMt[0:96, 0:1008])

            def ph2(b, jlist):
                s = st[b]
                if "pO" not in s:
                    s["pO"] = psO.tile([128, 512], mybir.dt.float32, tag="po",
                                       name=f"pO{b}")
                    s["pS"] = psO.tile([128, 512], mybir.dt.float32, tag="ps",
                                       name=f"pS{b}")
                pO, pS = s["pO"], s["pS"]
                for j in jlist:
                    gl0, w, l0 = _chunk_geo(j)
                    for h in range(4):
                        attoff, _ = _unit_off(j, h)
                        rhs = s["att"][0:96, attoff:attoff + 84] \
                            .rearrange("p (r c) -> p r c", c=14)[:, :, l0:l0 + w]
                        dstO = pO[32 * h:32 * h + 32, :NQ] \
                            .rearrange("p (r c) -> p r c", c=W2)[:, :, gl0:gl0 + w]
                        nc.tensor.matmul(
                            out=dstO,
                            lhsT=VTall[0:96, b * 768 + j * 128 + 32 * h:
                                       b * 768 + j * 128 + 32 * h + 32],
                            rhs=rhs, start=(j == 0), stop=(j == 5),
                            tile_position=(0, 32 * h))
                        dstS = pS[32 * h:32 * h + 32, :NQ] \
                            .rearrange("p (r c) -> p r c", c=W2)[:, :, gl0:gl0 + w]
                        nc.tensor.matmul(
                            out=dstS,
                            lhsT=MKt[0:96, (b * NCH + j) * 32:
                                     (b * NCH + j) * 32 + 32],
                            rhs=rhs, start=(j == 0), stop=(j == 5),
                            tile_position=(0, 32 * h))

            def norm(b):
                # 1/pS as exp(-ln pS) on ACT (Ln/Exp share one act table);
                # DVE reciprocal is ~1.9us, this is ~0.85us off-DVE.
                s = st[b]
                lnS = nrm.tile([128, NQ], mybir.dt.float32, tag="lns",
                               name=f"lnS{b}")
                nc.scalar.activation(out=lnS[:], in_=s["pS"][:, :NQ],
                                     func=mybir.ActivationFunctionType.Ln)
                rcpS = nrm.tile([128, NQ], mybir.dt.float32, tag="rcp",
                                name=f"rcpS{b}")
                nc.scalar.activation(out=rcpS[:], in_=lnS[:],
                                     func=mybir.ActivationFunctionType.Exp,
                                     scale=-1.0)
                onrm = nrm.tile([128, NQ], mybir.dt.bfloat16, tag="on",
                                name=f"on{b}")
                nc.vector.tensor_mul(out=onrm[:], in0=s["pO"][:, :NQ],
                                     in1=rcpS[:])
                s["on"] = onrm

            def final(b):
                s = st[b]
                pf = psP.tile([128, 512], mybir.dt.float32, tag="pp",
                              name=f"pf{b}")
                nc.tensor.matmul(out=pf[:, :NQ], lhsT=w_p[:], rhs=s["on"][:],
                                 start=True, stop=True)
                osb = nrm.tile([128, NQ], mybir.dt.float32, tag="osb",
                               name=f"osb{b}")
                nc.vector.tensor_copy(osb[:], pf[:, :NQ])
                nc.sync.dma_start(out=out_d[:, b * NQ:(b + 1) * NQ],
                                  in_=osb[:])
                st[b] = {}

            proj_all()
            if not _PIPE:
                for b in range(_NRUN):
                    if _STAGES >= 1.3:
                        ph1(b, 0)
                        ph1(b, 1)
                    if _STAGES >= 1.6:
                        expmask(b, 0)
                        expmask(b, 1)
                    if _STAGES >= 3:
                        ph2(b, [0, 1, 2])
                        ph2(b, [3, 4, 5])
                    if _STAGES >= 4:
                        norm(b)
                    if _STAGES >= 5:
                        final(b)
            else:
                # 3-stage software pipeline per iteration `it`:
                #   ph1/exp/mask(it) | ph2/norm(it-1) | final(it-2)
                for it in range(NBLK + 2):
                    if it < NBLK:
                        ph1(it, 0)
                        expmask(it, 0)
                        ph1(it, 1)
                        expmask(it, 1)
                    if 0 <= it - 1 < NBLK:
                        ph2(it - 1, [0, 1, 2])
                        ph2(it - 1, [3, 4, 5])
                        norm(it - 1)
                    if 0 <= it - 2 < NBLK:
                        final(it - 2)

    _split_multi_waits(nc)
    return nc


def _split_multi_waits(nc):
    """This walrus build rejects >1 sem wait per instruction: move extra
    waits onto dedicated single-wait NoOps inserted just before."""
    import copy
    from concourse import mybir

    tmpl = nc.sync.nop(nofuse=True, hint="wsplit_template").ins
    bb0 = nc.cur_bb.bb
    bb0.instructions = [i for i in bb0.instructions if i.name != tmpl.name]
    tmpl = copy.deepcopy(tmpl)

    ctr = 0
    for f in nc.m.functions:
        for bb in f.blocks:
            insts = list(bb.instructions)
            new, changed = [], False
            for inst in insts:
                si = getattr(inst, "sync_info", None)
                waits = list(si.on_wait) if si is not None and si.on_wait else []
                if len(waits) > 1:
                    for w in waits[:-1]:
                        ctr += 1
                        nop = copy.deepcopy(tmpl)
                        nop.name = f"I-wsplit{ctr}"
                        nop.engine = inst.engine
                        nop.sync_info = mybir.SyncInfo(on_wait=[w], on_update=[])
                        new.append(nop)
                    si.on_wait = [waits[-1]]
                    changed = True
                new.append(inst)
            if changed:
                bb.instructions = new


def _host_prep(x, m):
    """Per-core inputs: xq [128, NBLK*NQ] row-major center rows; xk/xm
    [128, NBLK*NK] chunk-major (key p = (j, kr, kc')); mk [128, NBLK*6*32]."""
    import ml_dtypes
    bf = ml_dtypes.bfloat16
    # chunk-major permutation of a 576-key block
    perm = np.array([kr * W2 + 8 * j + kc
                     for j in range(NCH) for kr in range(KR)
                     for kc in range(8)], np.int64)
    xqs, xks, xms, mks = [], [], [], []
    mf = (m > 0).astype(np.float32)
    for k in range(CORES):
        r0 = 12 * k - 6
        xpad = np.zeros((B, C, 24, W), np.float32)
        mpad = np.zeros((B, 1, 24, W), np.float32)
        lo, hi = max(0, r0), min(H, r0 + 24)
        xpad[:, :, lo - r0:hi - r0] = x[:, :, lo:hi]
        mpad[:, :, lo - r0:hi - r0] = mf[:, :, lo:hi]
        xmp = xpad * mpad

        def coset(t, ch):
            v = t.reshape(B, ch, KR, 2, W2, 2).transpose(1, 0, 3, 5, 2, 4)
            return v.reshape(ch, NBLK, NK)

        xc = coset(xpad, C)                       # [C, NBLK, NK] row-major
        xq = xc[:, :, 144:144 + NQ].reshape(C, NBLK * NQ)
        xk = xc[:, :, perm].reshape(C, NBLK * NK)
        xm = coset(xmp, C)[:, :, perm].reshape(C, NBLK * NK)
        xqs.append(np.ascontiguousarray(xq).astype(bf))
        xks.append(np.ascontiguousarray(xk).astype(bf))
        xms.append(np.ascontiguousarray(xm).astype(bf))
        mc = coset(mpad, 1)[0][:, perm].reshape(NBLK, NCH, 96)
        mk = np.zeros((128, NBLK * NCH * 32), np.float32)
        vals = np.where(mc > 0, 1.0, EPS)         # [NBLK, NCH, 96]
        mk[0:96] = np.repeat(vals.reshape(NBLK * NCH, 96).T, 32, axis=1)
        mks.append(mk.astype(bf))
    return xqs, xks, xms, mks


def _host_wm():
    """[128, 1008] bf16: 0/1 window mask, unit layout [12 units][6 qr][14 lc],
    key partition p = kr*8 + kc'."""
    import ml_dtypes
    kr = np.arange(KR)[:, None, None, None]
    kc = np.arange(8)[None, :, None, None]
    qr = np.arange(CR)[None, None, :, None]
    lc = np.arange(14)[None, None, None, :]
    win = ((kr - qr >= 0) & (kr - qr <= 6) & (lc >= kc) & (lc <= kc + 6))
    unit = win.reshape(96, 84).astype(np.float32)
    wm = np.zeros((128, 1008), np.float32)
    wm[0:96] = np.tile(unit, (1, 12))
    return wm.astype(ml_dtypes.bfloat16)


def _make_in_maps(x, m, Wq, Wk, Wv, Wp):
    import ml_dtypes
    bf = ml_dtypes.bfloat16
    xqs, xks, xms, mks = _host_prep(np.asarray(x, np.float32),
                                    np.asarray(m, np.int32))
    base = {
        "wm": _host_wm(),
        "wq": np.ascontiguousarray(np.asarray(Wq, np.float32).T).astype(bf),
        "wk": np.ascontiguousarray(np.asarray(Wk, np.float32).T).astype(bf),
        "wv": np.ascontiguousarray(np.asarray(Wv, np.float32).T).astype(bf),
        "wp": np.ascontiguousarray(np.asarray(Wp, np.float32).T).astype(bf),
    }
    return [{**base, "xq": xqs[k], "xk": xks[k], "xm": xms[k], "mk": mks[k]}
            for k in range(CORES)]


def kernel(x, m, Wq, Wk, Wv, Wp):
    global _prog
    from concourse.bass_utils import run_bass_kernel_spmd

    if _prog is None:
        _prog = _build_program()

    in_maps = _make_in_maps(x, m, Wq, Wk, Wv, Wp)
    res = run_bass_kernel_spmd(_prog, in_maps, list(range(CORES)))

    full = np.zeros((B, C, H, W), np.float32)
    for k in range(CORES):
        oc = res.results[k]["out"].reshape(C, B, 2, 2, CR, W2)
        o = oc.transpose(1, 0, 4, 2, 5, 3).reshape(B, C, 12, 96)
        full[:, :, 12 * k:12 * k + 12, :] = o
    return full
